# revision 15
# baseline (speedup 1.0000x reference)
"""Trainium2 Bass kernel for the ConvS2S-style decoder (nn_Decoder).

Strategy: pure data-parallel over batch — B=8 batch elements mapped 1:1 onto
8 NeuronCores, zero cross-core communication.  Host does the (tiny) embedding
gather + a few weight transposes; each core runs the full per-batch pipeline:

    u = emb @ W_e2h                                (H,T layout, left-pad cols = 1.0)
    6 x [ conv(K=3, via 3 shifted matmuls) -> GLU
          -> attention (energy in (S,T) layout, softmax over partition dim via
             ones-matmul column sums, normalization folded in as a rank-1
             broadcast matmul) -> residual updates ]
    convout = u.T @ W_h2e ; out = convout @ W_fc   (T,V layout, 64 V-chunks)

All matmuls run as float32r (fp32 storage, fp22 multiply) which is full PE
rate for moving-dim >= 256; every tensor feeding a matmul is typed float32r
end-to-end (walrus requires matmul inputs be produced pre-rounded).
Activations stay in SBUF for the whole layer stack; only weights stream from
HBM.
"""

import numpy as np
from contextlib import ExitStack

import bass_rust
import concourse.bass as bass
import concourse.mybir as mybir
import concourse.tile as tile
from concourse.alu_op_type import AluOpType

F32 = mybir.dt.float32
F32R = mybir.dt.float32r
AF = mybir.ActivationFunctionType
P = 128

_last_results = None


def _legalize_pe_waits(nc):
    """Walrus packs a self-loading (fp32/fp32r) Matmult's sync waits into the
    LDWEIGHTS hw descriptor, which has a single wait slot.  Move the waits of
    any multi-wait PE compute instruction onto EventSemaphore instructions
    (one wait each) inserted just before it on the PE queue — semantically
    identical wait point, but each carrier is within the hw limit."""
    n = 0
    absorb_types = (
        "InstMatmult",
        "InstLdweights",
        "InstDMACopy",
        "InstActivation",
        "InstTensorTensor",
        "InstTensorScalarPtr",
        "InstTensorCopy",
        "InstReciprocal",
        "InstMemset",
        "InstTensorReduce",
        "InstDrain",
    )
    for fn in nc.m.functions:
        for blk in fn.blocks:
            out = []
            changed = False
            for inst in blk.instructions:
                si = inst.sync_info
                if (
                    si is not None
                    and type(inst).__name__ in absorb_types
                ):
                    waits = list(si.on_wait)
                    if len(waits) > 1:
                        for w in waits:
                            out.append(
                                mybir.InstEventSemaphore(
                                    name=f"I-pewait{n}",
                                    engine=inst.engine,
                                    sync_info=bass_rust.SyncInfo(
                                        on_wait=[w], on_update=[]
                                    ),
                                    ins=[],
                                    outs=[],
                                )
                            )
                            n += 1
                        inst.sync_info = bass_rust.SyncInfo(
                            on_wait=[], on_update=list(si.on_update)
                        )
                        changed = True
                out.append(inst)
            if changed:
                blk.instructions = out
    return n


def build_decoder_nc(T, S, E, H, V, L, KW, CH, with_bias, pad_val=1.0, legalize=True):
    """Build the per-core Bass program.  All dims must be multiples of 128
    (except V which must be a multiple of CH, CH <= 512)."""
    kE, kH, kS, mT = E // P, H // P, S // P, T // P
    NCH = V // CH
    SQ = float(np.sqrt(np.float32(0.5)))
    S2 = 0.5  # SQ**2 exactly

    nc = bass.Bass()

    d_embT = nc.declare_dram_parameter("embT", [E, T], F32R, isOutput=False)
    d_embsT = nc.declare_dram_parameter("embsT", [E, T], F32, isOutput=False)
    d_encT = nc.declare_dram_parameter("encT", [E, S], F32R, isOutput=False)
    d_encC = nc.declare_dram_parameter("encC", [S, E], F32R, isOutput=False)
    d_we2h = nc.declare_dram_parameter("we2h", [E, H], F32R, isOutput=False)
    d_w1 = nc.declare_dram_parameter("w1", [H, E], F32R, isOutput=False)
    d_w2 = nc.declare_dram_parameter("w2", [E, H], F32R, isOutput=False)
    d_wh2e = nc.declare_dram_parameter("wh2e", [H, E], F32R, isOutput=False)
    d_fcw = nc.declare_dram_parameter("fcw", [E, V], F32R, isOutput=False)
    d_cw = nc.declare_dram_parameter("cw", [L, KW, H, 2 * H], F32R, isOutput=False)
    d_cpad = nc.declare_dram_parameter("c_pad", [P, KW - 1], F32R, isOutput=False)
    d_cones = nc.declare_dram_parameter("c_ones_col", [P, 1], F32R, isOutput=False)
    d_chalf = nc.declare_dram_parameter("c_halfones", [1, P], F32R, isOutput=False)
    if with_bias:
        d_be2h = nc.declare_dram_parameter("b_e2h", [1, H], F32R, isOutput=False)
        d_b1 = nc.declare_dram_parameter("b1", [1, E], F32R, isOutput=False)
        d_b2s2 = nc.declare_dram_parameter("b2s2", [1, H], F32R, isOutput=False)
        d_bh2e = nc.declare_dram_parameter("bh2e", [1, E], F32R, isOutput=False)
        d_cb = nc.declare_dram_parameter("cb", [L, 2 * H], F32R, isOutput=False)
    d_out = nc.declare_dram_parameter("out", [T, V], F32, isOutput=True)

    with tile.TileContext(nc) as tc, ExitStack() as ctx:
        pers = ctx.enter_context(tc.tile_pool(name="pers", bufs=1))
        pp = ctx.enter_context(tc.tile_pool(name="pp", bufs=8, space="PSUM"))

        # ---- persistent SBUF tensors -------------------------------------
        u = []
        for i in range(kH):
            t = pers.tile([P, T + KW - 1], F32R, tag=f"u{i}", name=f"u{i}")
            u.append(t)
            nc.sync.dma_start(t[:, 0 : KW - 1], d_cpad[:, :])
        embs_t = []
        for i in range(kE):
            t = pers.tile([P, T], F32, tag=f"embs{i}", name=f"embs{i}")
            nc.sync.dma_start(t, d_embsT[P * i : P * (i + 1), :])
            embs_t.append(t)
        encT_t = []
        for i in range(kE):
            t = pers.tile([P, S], F32R, tag=f"encT{i}", name=f"encTt{i}")
            nc.sync.dma_start(t, d_encT[P * i : P * (i + 1), :])
            encT_t.append(t)
        encC_t = []
        for i in range(kS):
            t = pers.tile([P, E], F32R, tag=f"encC{i}", name=f"encCt{i}")
            nc.sync.dma_start(t, d_encC[P * i : P * (i + 1), :])
            encC_t.append(t)
        w1_t = []
        for i in range(kH):
            t = pers.tile([P, E], F32R, tag=f"w1_{i}", name=f"w1t{i}")
            nc.sync.dma_start(t, d_w1[P * i : P * (i + 1), :])
            w1_t.append(t)
        w2_t = []
        for i in range(kE):
            t = pers.tile([P, H], F32R, tag=f"w2_{i}", name=f"w2t{i}")
            nc.sync.dma_start(t, d_w2[P * i : P * (i + 1), :])
            w2_t.append(t)
        ones_col = pers.tile([P, 1], F32R, tag="ones_col", name="ones_col")
        nc.sync.dma_start(ones_col, d_cones[:, :])
        halfones = pers.tile([1, P], F32R, tag="halfones", name="halfones")
        nc.sync.dma_start(halfones, d_chalf[:, :])

        if with_bias:
            d_crow = nc.declare_dram_parameter("c_ones_row", [1, T], F32R, isOutput=False)
            ones_row = pers.tile([1, T], F32R, tag="ones_row", name="ones_row")
            nc.sync.dma_start(ones_row, d_crow[:, :])
            be2h_t = pers.tile([1, H], F32R, tag="be2h", name="be2h_t")
            nc.sync.dma_start(be2h_t, d_be2h[:, :])
            b1_t = pers.tile([1, E], F32R, tag="b1", name="b1_t")
            nc.sync.dma_start(b1_t, d_b1[:, :])
            b2s2_t = pers.tile([1, H], F32R, tag="b2s2", name="b2s2_t")
            nc.sync.dma_start(b2s2_t, d_b2s2[:, :])
            bh2e_t = pers.tile([1, E], F32R, tag="bh2e", name="bh2e_t")
            nc.sync.dma_start(bh2e_t, d_bh2e[:, :])
            cb_t = []
            for l in range(L):
                t = pers.tile([1, 2 * H], F32R, tag=f"cb{l}", name=f"cb_t{l}")
                nc.sync.dma_start(t, d_cb[l : l + 1, :])
                cb_t.append(t)

        # ---- init: u[:, KW-1:] = W_e2h.T @ embT (+ b_e2h) ----------------
        with tc.tile_pool(name="initp", bufs=1) as initp:
            embT_t = []
            for i in range(kE):
                t = initp.tile([P, T], F32R, tag=f"embT{i}", name=f"embTt{i}")
                nc.sync.dma_start(t, d_embT[P * i : P * (i + 1), :])
                embT_t.append(t)
            we2h_t = []
            for i in range(kE):
                t = initp.tile([P, H], F32R, tag=f"we2h{i}", name=f"we2ht{i}")
                nc.sync.dma_start(t, d_we2h[P * i : P * (i + 1), :])
                we2h_t.append(t)
            for m in range(kH):
                ps = pp.tile([P, T], F32, tag="ps", name=f"initps{m}")
                for k in range(kE):
                    nc.tensor.matmul(
                        ps,
                        we2h_t[k][:, P * m : P * (m + 1)],
                        embT_t[k],
                        start=(k == 0),
                        stop=(k == kE - 1 and not with_bias),
                    )
                if with_bias:
                    nc.tensor.matmul(
                        ps,
                        be2h_t[:, P * m : P * (m + 1)],
                        ones_row,
                        start=False,
                        stop=True,
                    )
                nc.scalar.copy(u[m][:, KW - 1 :], ps)

        # ---- layer stack -------------------------------------------------
        with (
            tc.tile_pool(name="wconv_p", bufs=3) as wconv_p,
            tc.tile_pool(name="a_p", bufs=kH) as a_p,
            tc.tile_pool(name="sig_p", bufs=3) as sig_p,
            tc.tile_pool(name="glu_p", bufs=kH) as glu_p,
            tc.tile_pool(name="comb_p", bufs=kE) as comb_p,
            tc.tile_pool(name="ex_p", bufs=kS) as ex_p,
            tc.tile_pool(name="att_p", bufs=kE) as att_p,
            tc.tile_pool(name="rec_p", bufs=2) as rec_p,
            tc.tile_pool(name="y_p", bufs=3) as y_p,
        ):
            for l in range(L):
                # conv (2 halves: a-channels then g-channels) + GLU
                a_sb = []
                glu = []
                for half in range(2):
                    cps = [
                        pp.tile([P, T], F32, tag="ps", name=f"cps{l}_{half}_{m}")
                        for m in range(kH)
                    ]
                    n_mm = KW * kH
                    i_mm = 0
                    for kw in range(KW):
                        for k in range(kH):
                            wst = wconv_p.tile(
                                [P, H], F32R, tag="wst", name=f"wst{l}_{half}_{kw}_{k}"
                            )
                            nc.sync.dma_start(
                                wst,
                                d_cw[l, kw, P * k : P * (k + 1), H * half : H * (half + 1)],
                            )
                            for m in range(kH):
                                nc.tensor.matmul(
                                    cps[m],
                                    wst[:, P * m : P * (m + 1)],
                                    u[k][:, kw : kw + T],
                                    start=(i_mm == 0),
                                    stop=(i_mm == n_mm - 1 and not with_bias),
                                )
                            i_mm += 1
                    if with_bias:
                        for m in range(kH):
                            nc.tensor.matmul(
                                cps[m],
                                cb_t[l][
                                    :, half * H + P * m : half * H + P * (m + 1)
                                ],
                                ones_row,
                                start=False,
                                stop=True,
                            )
                    if half == 0:
                        for m in range(kH):
                            t = a_p.tile([P, T], F32, tag="a", name=f"asb{l}_{m}")
                            nc.scalar.copy(t, cps[m])
                            a_sb.append(t)
                    else:
                        for m in range(kH):
                            sg = sig_p.tile([P, T], F32, tag="sig", name=f"sig{l}_{m}")
                            nc.scalar.activation(sg, cps[m], AF.Sigmoid)
                            g = glu_p.tile([P, T], F32R, tag="glu", name=f"glu{l}_{m}")
                            nc.vector.tensor_mul(g, a_sb[m], sg)
                            glu.append(g)

                # attention: combined = (glu.T @ w1 (+b1)) * s + emb*s, (E,T)
                comb = []
                for m in range(kE):
                    ps = pp.tile([P, T], F32, tag="ps", name=f"ceps{l}_{m}")
                    for k in range(kH):
                        nc.tensor.matmul(
                            ps,
                            w1_t[k][:, P * m : P * (m + 1)],
                            glu[k],
                            start=(k == 0),
                            stop=(k == kH - 1 and not with_bias),
                        )
                    if with_bias:
                        nc.tensor.matmul(
                            ps,
                            b1_t[:, P * m : P * (m + 1)],
                            ones_row,
                            start=False,
                            stop=True,
                        )
                    c = comb_p.tile([P, T], F32R, tag="comb", name=f"comb{l}_{m}")
                    nc.vector.scalar_tensor_tensor(
                        c, ps, SQ, embs_t[m], AluOpType.mult, AluOpType.add
                    )
                    comb.append(c)

                # energy in (S, T) layout; exp elementwise (energies are
                # bounded ~|22| for this model, fp32-safe without max-sub)
                ex = []
                for m in range(kS):
                    ps = pp.tile([P, T], F32, tag="ps", name=f"enps{l}_{m}")
                    for k in range(kE):
                        nc.tensor.matmul(
                            ps,
                            encT_t[k][:, P * m : P * (m + 1)],
                            comb[k],
                            start=(k == 0),
                            stop=(k == kE - 1),
                        )
                    e = ex_p.tile([P, T], F32R, tag="ex", name=f"ex{l}_{m}")
                    nc.scalar.activation(e, ps, AF.Exp)
                    ex.append(e)

                # column sums over S (partition dim) via ones matmul; then
                # rec_bc[p, t] = 0.5 / sums[t] broadcast to 128 partitions
                sps = pp.tile([1, T], F32, tag="ps", name=f"sums{l}")
                for k in range(kS):
                    nc.tensor.matmul(
                        sps, ones_col, ex[k], start=(k == 0), stop=(k == kS - 1)
                    )
                rec32 = rec_p.tile([1, T], F32, tag="rec32", name=f"rec32_{l}")
                nc.vector.reciprocal(rec32, sps)
                rec = rec_p.tile([1, T], F32R, tag="rec", name=f"rec{l}")
                nc.scalar.copy(rec, rec32)
                bps = pp.tile([P, T], F32, tag="ps", name=f"bps{l}")
                nc.tensor.matmul(bps, halfones, rec, start=True, stop=True)
                rbc = rec_p.tile([P, T], F32, tag="rbc", name=f"rbc{l}")
                nc.vector.tensor_copy(rbc, bps)

                # attended (E,T), normalized and pre-scaled by s^2
                att = []
                for m in range(kE):
                    ps = pp.tile([P, T], F32, tag="ps", name=f"atps{l}_{m}")
                    for k in range(kS):
                        nc.tensor.matmul(
                            ps,
                            encC_t[k][:, P * m : P * (m + 1)],
                            ex[k],
                            start=(k == 0),
                            stop=(k == kS - 1),
                        )
                    a = att_p.tile([P, T], F32R, tag="att", name=f"att{l}_{m}")
                    nc.vector.tensor_mul(a, ps, rbc)
                    att.append(a)

                # att2 = w2.T @ att (+ b2*s^2); u = u*s + (glu*s^2 + att2)
                for m in range(kH):
                    ps = pp.tile([P, T], F32, tag="ps", name=f"a2ps{l}_{m}")
                    for k in range(kE):
                        nc.tensor.matmul(
                            ps,
                            w2_t[k][:, P * m : P * (m + 1)],
                            att[k],
                            start=(k == 0),
                            stop=(k == kE - 1 and not with_bias),
                        )
                    if with_bias:
                        nc.tensor.matmul(
                            ps,
                            b2s2_t[:, P * m : P * (m + 1)],
                            ones_row,
                            start=False,
                            stop=True,
                        )
                    y = y_p.tile([P, T], F32, tag="y", name=f"y{l}_{m}")
                    nc.vector.scalar_tensor_tensor(
                        y, glu[m], S2, ps, AluOpType.mult, AluOpType.add
                    )
                    nc.vector.scalar_tensor_tensor(
                        u[m][:, KW - 1 :],
                        u[m][:, KW - 1 :],
                        SQ,
                        y,
                        AluOpType.mult,
                        AluOpType.add,
                    )

        # ---- final: convout (E,T) then fc_out (T,V) ----------------------
        with (
            tc.tile_pool(name="wh2e_p", bufs=1) as wh2e_p,
            tc.tile_pool(name="co_p", bufs=1) as co_p,
            tc.tile_pool(name="fcw_p", bufs=2 * kE) as fcw_p,
            tc.tile_pool(name="ot_p", bufs=4) as ot_p,
        ):
            wh2e_t = []
            for i in range(kH):
                t = wh2e_p.tile([P, E], F32R, tag=f"wh2e{i}", name=f"wh2et{i}")
                nc.sync.dma_start(t, d_wh2e[P * i : P * (i + 1), :])
                wh2e_t.append(t)
            co = []
            for m in range(kE):
                ps = pp.tile([P, T], F32, tag="ps", name=f"cops{m}")
                for k in range(kH):
                    nc.tensor.matmul(
                        ps,
                        wh2e_t[k][:, P * m : P * (m + 1)],
                        u[k][:, KW - 1 :],
                        start=(k == 0),
                        stop=(k == kH - 1 and not with_bias),
                    )
                if with_bias:
                    nc.tensor.matmul(
                        ps,
                        bh2e_t[:, P * m : P * (m + 1)],
                        ones_row,
                        start=False,
                        stop=True,
                    )
                t = co_p.tile([P, T], F32R, tag=f"co{m}", name=f"co{m}")
                nc.scalar.copy(t, ps)
                co.append(t)

            for c in range(NCH):
                fts = []
                for k in range(kE):
                    ft = fcw_p.tile([P, CH], F32R, tag="fcw", name=f"fcw{c}_{k}")
                    nc.sync.dma_start(ft, d_fcw[P * k : P * (k + 1), CH * c : CH * (c + 1)])
                    fts.append(ft)
                for m in range(mT):
                    ps = pp.tile([P, CH], F32, tag="ps", name=f"fcps{c}_{m}")
                    for k in range(kE):
                        nc.tensor.matmul(
                            ps,
                            co[k][:, P * m : P * (m + 1)],
                            fts[k],
                            start=(k == 0),
                            stop=(k == kE - 1),
                        )
                    ot = ot_p.tile([P, CH], F32, tag="ot", name=f"ot{c}_{m}")
                    nc.scalar.copy(ot, ps)
                    nc.sync.dma_start(
                        d_out[P * m : P * (m + 1), CH * c : CH * (c + 1)], ot
                    )

    if legalize:
        _legalize_pe_waits(nc)
    return nc


def _host_prep(inp, T, L, KW):
    """Host-side input prep shared by kernel() and tests: embedding gather,
    transposes, conv-weight relayout."""
    f32 = np.float32
    trg = np.asarray(inp["trg"]).astype(np.int64)
    tok = np.asarray(inp["tok_emb"], dtype=f32)
    pos = np.asarray(inp["pos_emb"], dtype=f32)
    embedded = tok[trg] + pos[:T][None]  # (B,T,E)
    sq = f32(np.sqrt(np.float32(0.5)))
    embT = np.ascontiguousarray(embedded.transpose(0, 2, 1))
    embsT = np.ascontiguousarray((embedded * sq).transpose(0, 2, 1))
    encT = np.ascontiguousarray(
        np.asarray(inp["encoder_conved"], dtype=f32).transpose(0, 2, 1)
    )
    encC = np.ascontiguousarray(np.asarray(inp["encoder_combined"], dtype=f32))
    cw = np.ascontiguousarray(
        np.asarray(inp["conv_w"], dtype=f32).transpose(0, 3, 2, 1)
    )  # (L, KW, H, 2H)
    return embT, embsT, encT, encC, cw


def kernel(**inputs):
    B, T, S = 8, 512, 512
    E, H, V = 512, 1024, 32000
    KW, L = 3, 6
    CH = 500

    f32 = np.float32
    inp = {k: np.asarray(v) for k, v in inputs.items()}
    embT, embsT, encT, encC, cw = _host_prep(inp, T, L, KW)

    dev_biases = ["emb2hid_b", "conv_b", "attn_hid2emb_b", "attn_emb2hid_b", "hid2emb_b"]
    with_bias = any(np.any(np.asarray(inp[k])) for k in dev_biases)

    nc = build_decoder_nc(
        T=T, S=S, E=E, H=H, V=V, L=L, KW=KW, CH=CH, with_bias=with_bias
    )

    base = {
        "c_pad": np.full((128, KW - 1), f32(1.0)),
        "c_ones_col": np.ones((128, 1), f32),
        "c_halfones": np.full((1, 128), f32(0.5)),
        "we2h": np.ascontiguousarray(np.asarray(inp["emb2hid_w"], dtype=f32)),
        "w1": np.ascontiguousarray(np.asarray(inp["attn_hid2emb_w"], dtype=f32)),
        "w2": np.ascontiguousarray(np.asarray(inp["attn_emb2hid_w"], dtype=f32)),
        "wh2e": np.ascontiguousarray(np.asarray(inp["hid2emb_w"], dtype=f32)),
        "fcw": np.ascontiguousarray(np.asarray(inp["fc_out_w"], dtype=f32)),
        "cw": cw,
    }
    if with_bias:
        base |= {
            "c_ones_row": np.ones((1, T), f32),
            "b_e2h": np.asarray(inp["emb2hid_b"], dtype=f32).reshape(1, H),
            "b1": np.asarray(inp["attn_hid2emb_b"], dtype=f32).reshape(1, E),
            "b2s2": (np.asarray(inp["attn_emb2hid_b"], dtype=f32) * f32(0.5)).reshape(1, H),
            "bh2e": np.asarray(inp["hid2emb_b"], dtype=f32).reshape(1, E),
            "cb": np.ascontiguousarray(np.asarray(inp["conv_b"], dtype=f32)),
        }
    in_maps = [
        dict(base, embT=embT[c], embsT=embsT[c], encT=encT[c], encC=encC[c])
        for c in range(B)
    ]

    from concourse.bass_utils import run_bass_kernel_spmd

    import os

    trace = bool(os.environ.get("DECODER_TRACE"))
    res = run_bass_kernel_spmd(nc, in_maps, core_ids=list(range(B)), trace=trace)
    global _last_results
    _last_results = res
    out = np.stack([res.results[c]["out"] for c in range(B)]).astype(f32)

    fcb = np.asarray(inp["fc_out_b"], dtype=f32)
    if np.any(fcb):
        out = out + fcb[None, None, :]
    return out


# revision 26
# speedup vs baseline: 1.2821x; 1.2821x over previous
"""Trainium2 Bass kernel for the ConvS2S-style decoder (nn_Decoder).

Strategy: pure data-parallel over batch — B=8 batch elements mapped 1:1 onto
8 NeuronCores, zero cross-core communication.  Host does the (tiny) embedding
gather + a few weight transposes; each core runs the full per-batch pipeline:

    u = emb @ W_e2h                                (H,T layout, left-pad cols = 1.0)
    6 x [ conv(K=3, via 3 shifted matmuls) -> GLU
          -> attention (energy in (S,T) layout, softmax over partition dim via
             ones-matmul column sums, normalization folded in as a rank-1
             broadcast matmul) -> residual updates ]
    convout = u.T @ W_h2e ; out = convout @ W_fc   (T,V layout, 64 V-chunks)

All matmuls run as float32r (fp32 storage, fp22 multiply) which is full PE
rate for moving-dim >= 256; every tensor feeding a matmul is typed float32r
end-to-end (walrus requires matmul inputs be produced pre-rounded).
Activations stay in SBUF for the whole layer stack; only weights stream from
HBM.
"""

import numpy as np
from contextlib import ExitStack

import bass_rust
import concourse.bass as bass
import concourse.mybir as mybir
import concourse.tile as tile
from concourse.alu_op_type import AluOpType

F32 = mybir.dt.float32
F32R = mybir.dt.float32r
BF16 = mybir.dt.bfloat16
AF = mybir.ActivationFunctionType
P = 128

_last_results = None


def _legalize_pe_waits(nc):
    """Walrus packs a self-loading (fp32/fp32r) Matmult's sync waits into the
    LDWEIGHTS hw descriptor, which has a single wait slot.  Move the waits of
    any multi-wait PE compute instruction onto EventSemaphore instructions
    (one wait each) inserted just before it on the PE queue — semantically
    identical wait point, but each carrier is within the hw limit."""
    n = 0
    absorb_types = (
        "InstMatmult",
        "InstLdweights",
        "InstDMACopy",
        "InstActivation",
        "InstTensorTensor",
        "InstTensorScalarPtr",
        "InstTensorCopy",
        "InstReciprocal",
        "InstMemset",
        "InstTensorReduce",
        "InstDrain",
    )
    for fn in nc.m.functions:
        for blk in fn.blocks:
            out = []
            changed = False
            for inst in blk.instructions:
                si = inst.sync_info
                if (
                    si is not None
                    and type(inst).__name__ in absorb_types
                ):
                    waits = list(si.on_wait)
                    if len(waits) > 1:
                        for w in waits:
                            out.append(
                                mybir.InstEventSemaphore(
                                    name=f"I-pewait{n}",
                                    engine=inst.engine,
                                    sync_info=bass_rust.SyncInfo(
                                        on_wait=[w], on_update=[]
                                    ),
                                    ins=[],
                                    outs=[],
                                )
                            )
                            n += 1
                        inst.sync_info = bass_rust.SyncInfo(
                            on_wait=[], on_update=list(si.on_update)
                        )
                        changed = True
                out.append(inst)
            if changed:
                blk.instructions = out
    return n


def build_decoder_nc(T, S, E, H, V, L, KW, CH, with_bias, pad_val=1.0, legalize=True):
    """Build the per-core Bass program.  All dims must be multiples of 128
    (except V which must be a multiple of CH, CH <= 512)."""
    kE, kH, kS, mT = E // P, H // P, S // P, T // P
    NCH = V // CH
    SQ = float(np.sqrt(np.float32(0.5)))
    S2 = 0.5  # SQ**2 exactly

    nc = bass.Bass()

    d_embT = nc.declare_dram_parameter("embT", [E, T], F32R, isOutput=False)
    d_embsT = nc.declare_dram_parameter("embsT", [E, T], F32, isOutput=False)
    d_encT = nc.declare_dram_parameter("encT", [E, S], F32R, isOutput=False)
    d_encC = nc.declare_dram_parameter("encC", [S, E], F32R, isOutput=False)
    d_we2h = nc.declare_dram_parameter("we2h", [E, H], F32R, isOutput=False)
    d_w1 = nc.declare_dram_parameter("w1", [H, E], F32R, isOutput=False)
    d_w2 = nc.declare_dram_parameter("w2", [E, H], F32R, isOutput=False)
    d_wh2e = nc.declare_dram_parameter("wh2e", [H, E], F32R, isOutput=False)
    d_fcw = nc.declare_dram_parameter("fcw", [E, V], BF16, isOutput=False)
    d_cw = nc.declare_dram_parameter("cw", [L, KW, H, 2 * H], BF16, isOutput=False)
    d_cpad = nc.declare_dram_parameter("c_pad", [P, KW - 1], F32R, isOutput=False)
    d_cones = nc.declare_dram_parameter("c_ones_col", [P, 1], F32R, isOutput=False)
    d_chalf = nc.declare_dram_parameter("c_halfones", [1, P], F32R, isOutput=False)
    if with_bias:
        d_be2h = nc.declare_dram_parameter("b_e2h", [1, H], F32R, isOutput=False)
        d_b1 = nc.declare_dram_parameter("b1", [1, E], F32R, isOutput=False)
        d_b2s2 = nc.declare_dram_parameter("b2s2", [1, H], F32R, isOutput=False)
        d_bh2e = nc.declare_dram_parameter("bh2e", [1, E], F32R, isOutput=False)
    d_out = nc.declare_dram_parameter("out", [T, V], F32, isOutput=True)

    with tile.TileContext(nc) as tc, ExitStack() as ctx:
        pers = ctx.enter_context(tc.tile_pool(name="pers", bufs=1))
        pp = ctx.enter_context(tc.tile_pool(name="pp", bufs=8, space="PSUM"))

        # ---- persistent SBUF tensors -------------------------------------
        u = []
        for i in range(kH):
            t = pers.tile([P, T + KW - 1], F32R, tag=f"u{i}", name=f"u{i}")
            u.append(t)
            nc.sync.dma_start(t[:, 0 : KW - 1], d_cpad[:, :])
        embs_t = []
        for i in range(kE):
            t = pers.tile([P, T], F32, tag=f"embs{i}", name=f"embs{i}")
            nc.sync.dma_start(t, d_embsT[P * i : P * (i + 1), :])
            embs_t.append(t)
        encT_t = []
        for i in range(kE):
            t = pers.tile([P, S], F32R, tag=f"encT{i}", name=f"encTt{i}")
            nc.sync.dma_start(t, d_encT[P * i : P * (i + 1), :])
            encT_t.append(t)
        encC_t = []
        for i in range(kS):
            t = pers.tile([P, E], F32R, tag=f"encC{i}", name=f"encCt{i}")
            nc.sync.dma_start(t, d_encC[P * i : P * (i + 1), :])
            encC_t.append(t)
        w1_t = []
        for i in range(kH):
            t = pers.tile([P, E], F32R, tag=f"w1_{i}", name=f"w1t{i}")
            nc.sync.dma_start(t, d_w1[P * i : P * (i + 1), :])
            w1_t.append(t)
        w2_t = []
        for i in range(kE):
            t = pers.tile([P, H], F32R, tag=f"w2_{i}", name=f"w2t{i}")
            nc.sync.dma_start(t, d_w2[P * i : P * (i + 1), :])
            w2_t.append(t)
        ones_col = pers.tile([P, 1], F32R, tag="ones_col", name="ones_col")
        nc.sync.dma_start(ones_col, d_cones[:, :])
        halfones = pers.tile([1, P], F32R, tag="halfones", name="halfones")
        nc.sync.dma_start(halfones, d_chalf[:, :])

        if with_bias:
            d_crow = nc.declare_dram_parameter("c_ones_row", [1, T], F32R, isOutput=False)
            ones_row = pers.tile([1, T], F32R, tag="ones_row", name="ones_row")
            nc.sync.dma_start(ones_row, d_crow[:, :])
            ones_row_bf = pers.tile([1, T], BF16, tag="ones_row_bf", name="ones_row_bf")
            nc.vector.tensor_copy(ones_row_bf, ones_row)
            be2h_t = pers.tile([1, H], F32R, tag="be2h", name="be2h_t")
            nc.sync.dma_start(be2h_t, d_be2h[:, :])
            b1_t = pers.tile([1, E], F32R, tag="b1", name="b1_t")
            nc.sync.dma_start(b1_t, d_b1[:, :])
            b2s2_t = pers.tile([1, H], F32R, tag="b2s2", name="b2s2_t")
            nc.sync.dma_start(b2s2_t, d_b2s2[:, :])
            bh2e_t = pers.tile([1, E], F32R, tag="bh2e", name="bh2e_t")
            nc.sync.dma_start(bh2e_t, d_bh2e[:, :])
            d_cbf = nc.declare_dram_parameter("cb_bf", [L, 2 * H], BF16, isOutput=False)
            cb_t = []
            for l in range(L):
                t = pers.tile([1, 2 * H], BF16, tag=f"cb{l}", name=f"cb_t{l}")
                nc.sync.dma_start(t, d_cbf[l : l + 1, :])
                cb_t.append(t)

        # ---- init: u[:, KW-1:] = W_e2h.T @ embT (+ b_e2h) ----------------
        with tc.tile_pool(name="initp", bufs=1) as initp:
            embT_t = []
            for i in range(kE):
                t = initp.tile([P, T], F32R, tag=f"embT{i}", name=f"embTt{i}")
                nc.sync.dma_start(t, d_embT[P * i : P * (i + 1), :])
                embT_t.append(t)
            we2h_t = []
            for i in range(kE):
                t = initp.tile([P, H], F32R, tag=f"we2h{i}", name=f"we2ht{i}")
                nc.sync.dma_start(t, d_we2h[P * i : P * (i + 1), :])
                we2h_t.append(t)
            for m in range(kH):
                ps = pp.tile([P, T], F32, tag="ps", name=f"initps{m}")
                for k in range(kE):
                    nc.tensor.matmul(
                        ps,
                        we2h_t[k][:, P * m : P * (m + 1)],
                        embT_t[k],
                        start=(k == 0),
                        stop=(k == kE - 1 and not with_bias),
                    )
                if with_bias:
                    nc.tensor.matmul(
                        ps,
                        be2h_t[:, P * m : P * (m + 1)],
                        ones_row,
                        start=False,
                        stop=True,
                    )
                nc.scalar.copy(u[m][:, KW - 1 :], ps)

        # ---- layer stack -------------------------------------------------
        with (
            tc.tile_pool(name="ubf_p", bufs=kH) as ubf_p,
            tc.tile_pool(name="wconv_p", bufs=6) as wconv_p,
            tc.tile_pool(name="a_p", bufs=kH) as a_p,
            tc.tile_pool(name="sig_p", bufs=3) as sig_p,
            tc.tile_pool(name="glu_p", bufs=kH) as glu_p,
            tc.tile_pool(name="comb_p", bufs=kE) as comb_p,
            tc.tile_pool(name="ex_p", bufs=kS) as ex_p,
            tc.tile_pool(name="att_p", bufs=kE) as att_p,
            tc.tile_pool(name="rec_p", bufs=2) as rec_p,
            tc.tile_pool(name="y_p", bufs=3) as y_p,
        ):
            for l in range(L):
                # bf16 copy of the conv input (pad cols included) for the
                # bf16-weight conv matmuls
                u_bf = []
                for k in range(kH):
                    t = ubf_p.tile([P, T + KW - 1], BF16, tag="ubf", name=f"ubf{l}_{k}")
                    nc.vector.tensor_copy(t, u[k])
                    u_bf.append(t)
                # conv (2 halves: a-channels then g-channels) + GLU
                a_sb = []
                glu = []
                for half in range(2):
                    cps = [
                        pp.tile([P, T], F32, tag="ps", name=f"cps{l}_{half}_{m}")
                        for m in range(kH)
                    ]
                    n_mm = KW * kH
                    i_mm = 0
                    for kw in range(KW):
                        for k in range(kH):
                            wst = wconv_p.tile(
                                [P, H], BF16, tag="wst", name=f"wst{l}_{half}_{kw}_{k}"
                            )
                            nc.sync.dma_start(
                                wst,
                                d_cw[l, kw, P * k : P * (k + 1), H * half : H * (half + 1)],
                            )
                            for m in range(kH):
                                nc.tensor.matmul(
                                    cps[m],
                                    wst[:, P * m : P * (m + 1)],
                                    u_bf[k][:, kw : kw + T],
                                    start=(i_mm == 0),
                                    stop=(i_mm == n_mm - 1 and not with_bias),
                                )
                            i_mm += 1
                    if with_bias:
                        for m in range(kH):
                            nc.tensor.matmul(
                                cps[m],
                                cb_t[l][
                                    :, half * H + P * m : half * H + P * (m + 1)
                                ],
                                ones_row_bf,
                                start=False,
                                stop=True,
                            )
                    if half == 0:
                        for m in range(kH):
                            t = a_p.tile([P, T], F32, tag="a", name=f"asb{l}_{m}")
                            nc.scalar.copy(t, cps[m])
                            a_sb.append(t)
                    else:
                        for m in range(kH):
                            sg = sig_p.tile([P, T], F32, tag="sig", name=f"sig{l}_{m}")
                            nc.scalar.activation(sg, cps[m], AF.Sigmoid)
                            g = glu_p.tile([P, T], F32R, tag="glu", name=f"glu{l}_{m}")
                            nc.vector.tensor_mul(g, a_sb[m], sg)
                            glu.append(g)

                # attention: combined = (glu.T @ w1 (+b1)) * s + emb*s, (E,T)
                comb = []
                for m in range(kE):
                    ps = pp.tile([P, T], F32, tag="ps", name=f"ceps{l}_{m}")
                    for k in range(kH):
                        nc.tensor.matmul(
                            ps,
                            w1_t[k][:, P * m : P * (m + 1)],
                            glu[k],
                            start=(k == 0),
                            stop=(k == kH - 1 and not with_bias),
                        )
                    if with_bias:
                        nc.tensor.matmul(
                            ps,
                            b1_t[:, P * m : P * (m + 1)],
                            ones_row,
                            start=False,
                            stop=True,
                        )
                    c = comb_p.tile([P, T], F32R, tag="comb", name=f"comb{l}_{m}")
                    nc.vector.scalar_tensor_tensor(
                        c, ps, SQ, embs_t[m], AluOpType.mult, AluOpType.add
                    )
                    comb.append(c)

                # energy in (S, T) layout; exp elementwise (energies are
                # bounded ~|22| for this model, fp32-safe without max-sub)
                ex = []
                for m in range(kS):
                    ps = pp.tile([P, T], F32, tag="ps", name=f"enps{l}_{m}")
                    for k in range(kE):
                        nc.tensor.matmul(
                            ps,
                            encT_t[k][:, P * m : P * (m + 1)],
                            comb[k],
                            start=(k == 0),
                            stop=(k == kE - 1),
                        )
                    e = ex_p.tile([P, T], F32R, tag="ex", name=f"ex{l}_{m}")
                    nc.scalar.activation(e, ps, AF.Exp)
                    ex.append(e)

                # column sums over S (partition dim) via ones matmul; then
                # rec_bc[p, t] = 0.5 / sums[t] broadcast to 128 partitions
                sps = pp.tile([1, T], F32, tag="ps", name=f"sums{l}")
                for k in range(kS):
                    nc.tensor.matmul(
                        sps, ones_col, ex[k], start=(k == 0), stop=(k == kS - 1)
                    )
                rec32 = rec_p.tile([1, T], F32, tag="rec32", name=f"rec32_{l}")
                nc.vector.reciprocal(rec32, sps)
                rec = rec_p.tile([1, T], F32R, tag="rec", name=f"rec{l}")
                nc.scalar.copy(rec, rec32)
                bps = pp.tile([P, T], F32, tag="ps", name=f"bps{l}")
                nc.tensor.matmul(bps, halfones, rec, start=True, stop=True)
                rbc = rec_p.tile([P, T], F32, tag="rbc", name=f"rbc{l}")
                nc.vector.tensor_copy(rbc, bps)

                # attended (E,T), normalized and pre-scaled by s^2
                att = []
                for m in range(kE):
                    ps = pp.tile([P, T], F32, tag="ps", name=f"atps{l}_{m}")
                    for k in range(kS):
                        nc.tensor.matmul(
                            ps,
                            encC_t[k][:, P * m : P * (m + 1)],
                            ex[k],
                            start=(k == 0),
                            stop=(k == kS - 1),
                        )
                    a = att_p.tile([P, T], F32R, tag="att", name=f"att{l}_{m}")
                    nc.vector.tensor_mul(a, ps, rbc)
                    att.append(a)

                # att2 = w2.T @ att (+ b2*s^2); u = u*s + (glu*s^2 + att2)
                for m in range(kH):
                    ps = pp.tile([P, T], F32, tag="ps", name=f"a2ps{l}_{m}")
                    for k in range(kE):
                        nc.tensor.matmul(
                            ps,
                            w2_t[k][:, P * m : P * (m + 1)],
                            att[k],
                            start=(k == 0),
                            stop=(k == kE - 1 and not with_bias),
                        )
                    if with_bias:
                        nc.tensor.matmul(
                            ps,
                            b2s2_t[:, P * m : P * (m + 1)],
                            ones_row,
                            start=False,
                            stop=True,
                        )
                    y = y_p.tile([P, T], F32, tag="y", name=f"y{l}_{m}")
                    nc.vector.scalar_tensor_tensor(
                        y, glu[m], S2, ps, AluOpType.mult, AluOpType.add
                    )
                    nc.vector.scalar_tensor_tensor(
                        u[m][:, KW - 1 :],
                        u[m][:, KW - 1 :],
                        SQ,
                        y,
                        AluOpType.mult,
                        AluOpType.add,
                    )

        # ---- final: convout (E,T) then fc_out (T,V) ----------------------
        with (
            tc.tile_pool(name="wh2e_p", bufs=1) as wh2e_p,
            tc.tile_pool(name="co_p", bufs=1) as co_p,
            tc.tile_pool(name="fcw_p", bufs=2 * kE) as fcw_p,
            tc.tile_pool(name="ot_p", bufs=4) as ot_p,
        ):
            wh2e_t = []
            for i in range(kH):
                t = wh2e_p.tile([P, E], F32R, tag=f"wh2e{i}", name=f"wh2et{i}")
                nc.sync.dma_start(t, d_wh2e[P * i : P * (i + 1), :])
                wh2e_t.append(t)
            co = []
            for m in range(kE):
                ps = pp.tile([P, T], F32, tag="ps", name=f"cops{m}")
                for k in range(kH):
                    nc.tensor.matmul(
                        ps,
                        wh2e_t[k][:, P * m : P * (m + 1)],
                        u[k][:, KW - 1 :],
                        start=(k == 0),
                        stop=(k == kH - 1 and not with_bias),
                    )
                if with_bias:
                    nc.tensor.matmul(
                        ps,
                        bh2e_t[:, P * m : P * (m + 1)],
                        ones_row,
                        start=False,
                        stop=True,
                    )
                t = co_p.tile([P, T], BF16, tag=f"co{m}", name=f"co{m}")
                nc.scalar.copy(t, ps)
                co.append(t)

            for c in range(NCH):
                fts = []
                for k in range(kE):
                    ft = fcw_p.tile([P, CH], BF16, tag="fcw", name=f"fcw{c}_{k}")
                    nc.sync.dma_start(ft, d_fcw[P * k : P * (k + 1), CH * c : CH * (c + 1)])
                    fts.append(ft)
                for m in range(mT):
                    ps = pp.tile([P, CH], F32, tag="ps", name=f"fcps{c}_{m}")
                    for k in range(kE):
                        nc.tensor.matmul(
                            ps,
                            co[k][:, P * m : P * (m + 1)],
                            fts[k],
                            start=(k == 0),
                            stop=(k == kE - 1),
                        )
                    ot = ot_p.tile([P, CH], F32, tag="ot", name=f"ot{c}_{m}")
                    nc.scalar.copy(ot, ps)
                    nc.sync.dma_start(
                        d_out[P * m : P * (m + 1), CH * c : CH * (c + 1)], ot
                    )

    if legalize:
        _legalize_pe_waits(nc)
    return nc


def _host_prep(inp, T, L, KW):
    """Host-side input prep shared by kernel() and tests: embedding gather,
    transposes, conv-weight relayout."""
    f32 = np.float32
    trg = np.asarray(inp["trg"]).astype(np.int64)
    tok = np.asarray(inp["tok_emb"], dtype=f32)
    pos = np.asarray(inp["pos_emb"], dtype=f32)
    embedded = tok[trg] + pos[:T][None]  # (B,T,E)
    sq = f32(np.sqrt(np.float32(0.5)))
    embT = np.ascontiguousarray(embedded.transpose(0, 2, 1))
    embsT = np.ascontiguousarray((embedded * sq).transpose(0, 2, 1))
    encT = np.ascontiguousarray(
        np.asarray(inp["encoder_conved"], dtype=f32).transpose(0, 2, 1)
    )
    encC = np.ascontiguousarray(np.asarray(inp["encoder_combined"], dtype=f32))
    import ml_dtypes

    cw = np.ascontiguousarray(
        np.asarray(inp["conv_w"], dtype=f32).transpose(0, 3, 2, 1)
    ).astype(ml_dtypes.bfloat16)  # (L, KW, H, 2H) bf16
    return embT, embsT, encT, encC, cw


def kernel(**inputs):
    B, T, S = 8, 512, 512
    E, H, V = 512, 1024, 32000
    KW, L = 3, 6
    CH = 500

    import ml_dtypes

    f32 = np.float32
    inp = {k: np.asarray(v) for k, v in inputs.items()}
    embT, embsT, encT, encC, cw = _host_prep(inp, T, L, KW)

    dev_biases = ["emb2hid_b", "conv_b", "attn_hid2emb_b", "attn_emb2hid_b", "hid2emb_b"]
    with_bias = any(np.any(np.asarray(inp[k])) for k in dev_biases)

    nc = build_decoder_nc(
        T=T, S=S, E=E, H=H, V=V, L=L, KW=KW, CH=CH, with_bias=with_bias
    )

    base = {
        "c_pad": np.full((128, KW - 1), f32(1.0)),
        "c_ones_col": np.ones((128, 1), f32),
        "c_halfones": np.full((1, 128), f32(0.5)),
        "we2h": np.ascontiguousarray(np.asarray(inp["emb2hid_w"], dtype=f32)),
        "w1": np.ascontiguousarray(np.asarray(inp["attn_hid2emb_w"], dtype=f32)),
        "w2": np.ascontiguousarray(np.asarray(inp["attn_emb2hid_w"], dtype=f32)),
        "wh2e": np.ascontiguousarray(np.asarray(inp["hid2emb_w"], dtype=f32)),
        "fcw": np.ascontiguousarray(np.asarray(inp["fc_out_w"], dtype=f32)).astype(
            ml_dtypes.bfloat16
        ),
        "cw": cw,
    }
    if with_bias:
        base |= {
            "c_ones_row": np.ones((1, T), f32),
            "b_e2h": np.asarray(inp["emb2hid_b"], dtype=f32).reshape(1, H),
            "b1": np.asarray(inp["attn_hid2emb_b"], dtype=f32).reshape(1, E),
            "b2s2": (np.asarray(inp["attn_emb2hid_b"], dtype=f32) * f32(0.5)).reshape(1, H),
            "bh2e": np.asarray(inp["hid2emb_b"], dtype=f32).reshape(1, E),
            "cb_bf": np.ascontiguousarray(np.asarray(inp["conv_b"], dtype=f32)).astype(
                ml_dtypes.bfloat16
            ),
        }
    in_maps = [
        dict(base, embT=embT[c], embsT=embsT[c], encT=encT[c], encC=encC[c])
        for c in range(B)
    ]

    from concourse.bass_utils import run_bass_kernel_spmd

    import os

    trace = bool(os.environ.get("DECODER_TRACE"))
    res = run_bass_kernel_spmd(nc, in_maps, core_ids=list(range(B)), trace=trace)
    global _last_results
    _last_results = res
    out = np.stack([res.results[c]["out"] for c in range(B)]).astype(f32)

    fcb = np.asarray(inp["fc_out_b"], dtype=f32)
    if np.any(fcb):
        out = out + fcb[None, None, :]
    return out


# revision 36
# speedup vs baseline: 1.3235x; 1.0323x over previous
"""Trainium2 Bass kernel for the ConvS2S-style decoder (nn_Decoder).

Strategy: pure data-parallel over batch — B=8 batch elements mapped 1:1 onto
8 NeuronCores, zero cross-core communication.  Host does the (tiny) embedding
gather + a few weight transposes; each core runs the full per-batch pipeline:

    u = emb @ W_e2h                                (H,T layout, left-pad cols = 1.0)
    6 x [ conv(K=3, via 3 shifted matmuls) -> GLU
          -> attention (energy in (S,T) layout, softmax over partition dim via
             ones-matmul column sums, normalization folded in as a rank-1
             broadcast matmul) -> residual updates ]
    convout = u.T @ W_h2e ; out = convout @ W_fc   (T,V layout, 64 V-chunks)

All matmuls run as float32r (fp32 storage, fp22 multiply) which is full PE
rate for moving-dim >= 256; every tensor feeding a matmul is typed float32r
end-to-end (walrus requires matmul inputs be produced pre-rounded).
Activations stay in SBUF for the whole layer stack; only weights stream from
HBM.
"""

import numpy as np
from contextlib import ExitStack

import bass_rust
import concourse.bass as bass
import concourse.mybir as mybir
import concourse.tile as tile
from concourse.alu_op_type import AluOpType

F32 = mybir.dt.float32
F32R = mybir.dt.float32r
BF16 = mybir.dt.bfloat16
AF = mybir.ActivationFunctionType
P = 128

_last_results = None


def _legalize_pe_waits(nc):
    """Walrus packs a self-loading (fp32/fp32r) Matmult's sync waits into the
    LDWEIGHTS hw descriptor, which has a single wait slot.  Move the waits of
    any multi-wait PE compute instruction onto EventSemaphore instructions
    (one wait each) inserted just before it on the PE queue — semantically
    identical wait point, but each carrier is within the hw limit."""
    n = 0
    absorb_types = (
        "InstMatmult",
        "InstLdweights",
        "InstDMACopy",
        "InstActivation",
        "InstTensorTensor",
        "InstTensorScalarPtr",
        "InstTensorCopy",
        "InstReciprocal",
        "InstMemset",
        "InstTensorReduce",
        "InstDrain",
    )
    for fn in nc.m.functions:
        for blk in fn.blocks:
            out = []
            changed = False
            for inst in blk.instructions:
                si = inst.sync_info
                if (
                    si is not None
                    and type(inst).__name__ in absorb_types
                ):
                    waits = list(si.on_wait)
                    if len(waits) > 1:
                        for w in waits:
                            out.append(
                                mybir.InstEventSemaphore(
                                    name=f"I-pewait{n}",
                                    engine=inst.engine,
                                    sync_info=bass_rust.SyncInfo(
                                        on_wait=[w], on_update=[]
                                    ),
                                    ins=[],
                                    outs=[],
                                )
                            )
                            n += 1
                        inst.sync_info = bass_rust.SyncInfo(
                            on_wait=[], on_update=list(si.on_update)
                        )
                        changed = True
                out.append(inst)
            if changed:
                blk.instructions = out
    return n


def build_decoder_nc(T, S, E, H, V, L, KW, CH, with_bias, pad_val=1.0, legalize=True):
    """Build the per-core Bass program.  All dims must be multiples of 128
    (except V which must be a multiple of CH, CH <= 512)."""
    kE, kH, kS, mT = E // P, H // P, S // P, T // P
    NCH = V // CH
    SQ = float(np.sqrt(np.float32(0.5)))
    S2 = 0.5  # SQ**2 exactly

    nc = bass.Bass()

    d_embT = nc.declare_dram_parameter("embT", [E, T], F32R, isOutput=False)
    d_embsT = nc.declare_dram_parameter("embsT", [E, T], F32, isOutput=False)
    d_encT = nc.declare_dram_parameter("encT", [E, S], F32R, isOutput=False)
    d_encC = nc.declare_dram_parameter("encC", [S, E], F32R, isOutput=False)
    d_we2h = nc.declare_dram_parameter("we2h", [E, H], F32R, isOutput=False)
    d_w1 = nc.declare_dram_parameter("w1", [H, E], F32R, isOutput=False)
    d_w2 = nc.declare_dram_parameter("w2", [E, H], F32R, isOutput=False)
    d_wh2e = nc.declare_dram_parameter("wh2e", [H, E], F32R, isOutput=False)
    d_fcw = nc.declare_dram_parameter("fcw", [E, V], BF16, isOutput=False)
    d_cw = nc.declare_dram_parameter("cw", [L, KW, H, 2 * H], BF16, isOutput=False)
    d_cpad = nc.declare_dram_parameter("c_pad", [P, KW - 1], F32R, isOutput=False)
    d_cones = nc.declare_dram_parameter("c_ones_col", [P, 1], F32R, isOutput=False)
    d_chalf = nc.declare_dram_parameter("c_halfones", [1, P], F32R, isOutput=False)
    if with_bias:
        d_be2h = nc.declare_dram_parameter("b_e2h", [1, H], F32R, isOutput=False)
        d_b1 = nc.declare_dram_parameter("b1", [1, E], F32R, isOutput=False)
        d_b2s2 = nc.declare_dram_parameter("b2s2", [1, H], F32R, isOutput=False)
        d_bh2e = nc.declare_dram_parameter("bh2e", [1, E], F32R, isOutput=False)
    d_out = nc.declare_dram_parameter("out", [T, V], F32, isOutput=True)

    with tile.TileContext(nc) as tc, ExitStack() as ctx:
        pers = ctx.enter_context(tc.tile_pool(name="pers", bufs=1))
        pp = ctx.enter_context(tc.tile_pool(name="pp", bufs=8, space="PSUM"))

        # ---- persistent SBUF tensors -------------------------------------
        u = []
        for i in range(kH):
            t = pers.tile([P, T + KW - 1], F32R, tag=f"u{i}", name=f"u{i}")
            u.append(t)
            nc.sync.dma_start(t[:, 0 : KW - 1], d_cpad[:, :])
        embs_t = []
        for i in range(kE):
            t = pers.tile([P, T], F32, tag=f"embs{i}", name=f"embs{i}")
            nc.sync.dma_start(t, d_embsT[P * i : P * (i + 1), :])
            embs_t.append(t)
        encT_t = []
        for i in range(kE):
            t = pers.tile([P, S], F32R, tag=f"encT{i}", name=f"encTt{i}")
            nc.sync.dma_start(t, d_encT[P * i : P * (i + 1), :])
            encT_t.append(t)
        encC_t = []
        for i in range(kS):
            t = pers.tile([P, E], F32R, tag=f"encC{i}", name=f"encCt{i}")
            nc.sync.dma_start(t, d_encC[P * i : P * (i + 1), :])
            encC_t.append(t)
        w1_t = []
        for i in range(kH):
            t = pers.tile([P, E], F32R, tag=f"w1_{i}", name=f"w1t{i}")
            nc.sync.dma_start(t, d_w1[P * i : P * (i + 1), :])
            w1_t.append(t)
        w2_t = []
        for i in range(kE):
            t = pers.tile([P, H], F32R, tag=f"w2_{i}", name=f"w2t{i}")
            nc.sync.dma_start(t, d_w2[P * i : P * (i + 1), :])
            w2_t.append(t)
        ones_col = pers.tile([P, 1], F32R, tag="ones_col", name="ones_col")
        nc.sync.dma_start(ones_col, d_cones[:, :])
        halfones = pers.tile([1, P], F32R, tag="halfones", name="halfones")
        nc.sync.dma_start(halfones, d_chalf[:, :])

        if with_bias:
            d_crow = nc.declare_dram_parameter("c_ones_row", [1, T], F32R, isOutput=False)
            ones_row = pers.tile([1, T], F32R, tag="ones_row", name="ones_row")
            nc.sync.dma_start(ones_row, d_crow[:, :])
            ones_row_bf = pers.tile([1, T], BF16, tag="ones_row_bf", name="ones_row_bf")
            nc.vector.tensor_copy(ones_row_bf, ones_row)
            be2h_t = pers.tile([1, H], F32R, tag="be2h", name="be2h_t")
            nc.sync.dma_start(be2h_t, d_be2h[:, :])
            b1_t = pers.tile([1, E], F32R, tag="b1", name="b1_t")
            nc.sync.dma_start(b1_t, d_b1[:, :])
            b2s2_t = pers.tile([1, H], F32R, tag="b2s2", name="b2s2_t")
            nc.sync.dma_start(b2s2_t, d_b2s2[:, :])
            bh2e_t = pers.tile([1, E], F32R, tag="bh2e", name="bh2e_t")
            nc.sync.dma_start(bh2e_t, d_bh2e[:, :])
            d_cbf = nc.declare_dram_parameter("cb_bf", [L, 2 * H], BF16, isOutput=False)
            cb_t = []
            for l in range(L):
                t = pers.tile([1, 2 * H], BF16, tag=f"cb{l}", name=f"cb_t{l}")
                nc.sync.dma_start(t, d_cbf[l : l + 1, :])
                cb_t.append(t)

        # ---- init: u[:, KW-1:] = W_e2h.T @ embT (+ b_e2h) ----------------
        with tc.tile_pool(name="initp", bufs=1) as initp:
            embT_t = []
            for i in range(kE):
                t = initp.tile([P, T], F32R, tag=f"embT{i}", name=f"embTt{i}")
                nc.sync.dma_start(t, d_embT[P * i : P * (i + 1), :])
                embT_t.append(t)
            we2h_t = []
            for i in range(kE):
                t = initp.tile([P, H], F32R, tag=f"we2h{i}", name=f"we2ht{i}")
                nc.sync.dma_start(t, d_we2h[P * i : P * (i + 1), :])
                we2h_t.append(t)
            for m in range(kH):
                ps = pp.tile([P, T], F32, tag="ps", name=f"initps{m}")
                for k in range(kE):
                    nc.tensor.matmul(
                        ps,
                        we2h_t[k][:, P * m : P * (m + 1)],
                        embT_t[k],
                        start=(k == 0),
                        stop=(k == kE - 1 and not with_bias),
                    )
                if with_bias:
                    nc.tensor.matmul(
                        ps,
                        be2h_t[:, P * m : P * (m + 1)],
                        ones_row,
                        start=False,
                        stop=True,
                    )
                nc.scalar.copy(u[m][:, KW - 1 :], ps)

        # ---- layer stack -------------------------------------------------
        n_stripes = KW * kH
        with (
            tc.tile_pool(name="ubf_p", bufs=kH) as ubf_p,
            tc.tile_pool(name="wconv_p", bufs=n_stripes + 8) as wconv_p,
            tc.tile_pool(name="sig_p", bufs=kH) as sig_p,
            tc.tile_pool(name="glu_p", bufs=kH) as glu_p,
            tc.tile_pool(name="comb_p", bufs=kE) as comb_p,
            tc.tile_pool(name="ex_p", bufs=kS) as ex_p,
            tc.tile_pool(name="att_p", bufs=kE) as att_p,
            tc.tile_pool(name="rec_p", bufs=1) as rec_p,
            tc.tile_pool(name="y_p", bufs=2) as y_p,
        ):
            for l in range(L):
                # bf16 copy of the conv input (pad cols included) for the
                # bf16-weight conv matmuls
                u_bf = []
                for k in range(kH):
                    t = ubf_p.tile([P, T + KW - 1], BF16, tag="ubf", name=f"ubf{l}_{k}")
                    nc.vector.tensor_copy(t, u[k])
                    u_bf.append(t)
                # conv + GLU: g-half (gate) first, then a-half; m-outer with
                # the half's full weight set resident so each psum finishes
                # early and GLU/attention overlap the remaining conv matmuls.
                sig = []
                glu = []
                for half in (1, 0):  # 1 = gate channels [H:2H), 0 = a [0:H)
                    wsts = []
                    for kw in range(KW):
                        for k in range(kH):
                            wst = wconv_p.tile(
                                [P, H], BF16, tag="wst", name=f"wst{l}_{half}_{kw}_{k}"
                            )
                            nc.sync.dma_start(
                                wst,
                                d_cw[l, kw, P * k : P * (k + 1), H * half : H * (half + 1)],
                            )
                            wsts.append((kw, k, wst))
                    for m in range(kH):
                        cps = pp.tile([P, T], F32, tag="ps", name=f"cps{l}_{half}_{m}")
                        for i_mm, (kw, k, wst) in enumerate(wsts):
                            nc.tensor.matmul(
                                cps,
                                wst[:, P * m : P * (m + 1)],
                                u_bf[k][:, kw : kw + T],
                                start=(i_mm == 0),
                                stop=(i_mm == n_stripes - 1 and not with_bias),
                            )
                        if with_bias:
                            nc.tensor.matmul(
                                cps,
                                cb_t[l][
                                    :, half * H + P * m : half * H + P * (m + 1)
                                ],
                                ones_row_bf,
                                start=False,
                                stop=True,
                            )
                        if half == 1:
                            sg = sig_p.tile([P, T], BF16, tag="sig", name=f"sig{l}_{m}")
                            nc.scalar.activation(sg, cps, AF.Sigmoid)
                            sig.append(sg)
                        else:
                            g = glu_p.tile([P, T], F32R, tag="glu", name=f"glu{l}_{m}")
                            nc.vector.tensor_mul(g, cps, sig[m])
                            glu.append(g)

                # attention: combined = (glu.T @ w1 (+b1)) * s + emb*s, (E,T)
                comb = []
                for m in range(kE):
                    ps = pp.tile([P, T], F32, tag="ps", name=f"ceps{l}_{m}")
                    for k in range(kH):
                        nc.tensor.matmul(
                            ps,
                            w1_t[k][:, P * m : P * (m + 1)],
                            glu[k],
                            start=(k == 0),
                            stop=(k == kH - 1 and not with_bias),
                        )
                    if with_bias:
                        nc.tensor.matmul(
                            ps,
                            b1_t[:, P * m : P * (m + 1)],
                            ones_row,
                            start=False,
                            stop=True,
                        )
                    c = comb_p.tile([P, T], F32R, tag="comb", name=f"comb{l}_{m}")
                    nc.vector.scalar_tensor_tensor(
                        c, ps, SQ, embs_t[m], AluOpType.mult, AluOpType.add
                    )
                    comb.append(c)

                # energy in (S, T) layout; exp elementwise (energies are
                # bounded ~|22| for this model, fp32-safe without max-sub)
                ex = []
                for m in range(kS):
                    ps = pp.tile([P, T], F32, tag="ps", name=f"enps{l}_{m}")
                    for k in range(kE):
                        nc.tensor.matmul(
                            ps,
                            encT_t[k][:, P * m : P * (m + 1)],
                            comb[k],
                            start=(k == 0),
                            stop=(k == kE - 1),
                        )
                    e = ex_p.tile([P, T], F32R, tag="ex", name=f"ex{l}_{m}")
                    nc.scalar.activation(e, ps, AF.Exp)
                    ex.append(e)

                # column sums over S (partition dim) via ones matmul; then
                # rec_bc[p, t] = 0.5 / sums[t] broadcast to 128 partitions
                sps = pp.tile([1, T], F32, tag="ps", name=f"sums{l}")
                for k in range(kS):
                    nc.tensor.matmul(
                        sps, ones_col, ex[k], start=(k == 0), stop=(k == kS - 1)
                    )
                rec32 = rec_p.tile([1, T], F32, tag="rec32", name=f"rec32_{l}")
                nc.vector.reciprocal(rec32, sps)
                rec = rec_p.tile([1, T], F32R, tag="rec", name=f"rec{l}")
                nc.scalar.copy(rec, rec32)
                bps = pp.tile([P, T], F32, tag="ps", name=f"bps{l}")
                nc.tensor.matmul(bps, halfones, rec, start=True, stop=True)
                rbc = rec_p.tile([P, T], F32, tag="rbc", name=f"rbc{l}")
                nc.vector.tensor_copy(rbc, bps)

                # attended (E,T), normalized and pre-scaled by s^2
                att = []
                for m in range(kE):
                    ps = pp.tile([P, T], F32, tag="ps", name=f"atps{l}_{m}")
                    for k in range(kS):
                        nc.tensor.matmul(
                            ps,
                            encC_t[k][:, P * m : P * (m + 1)],
                            ex[k],
                            start=(k == 0),
                            stop=(k == kS - 1),
                        )
                    a = att_p.tile([P, T], F32R, tag="att", name=f"att{l}_{m}")
                    nc.vector.tensor_mul(a, ps, rbc)
                    att.append(a)

                # att2 = w2.T @ att (+ b2*s^2); u = u*s + (glu*s^2 + att2)
                for m in range(kH):
                    ps = pp.tile([P, T], F32, tag="ps", name=f"a2ps{l}_{m}")
                    for k in range(kE):
                        nc.tensor.matmul(
                            ps,
                            w2_t[k][:, P * m : P * (m + 1)],
                            att[k],
                            start=(k == 0),
                            stop=(k == kE - 1 and not with_bias),
                        )
                    if with_bias:
                        nc.tensor.matmul(
                            ps,
                            b2s2_t[:, P * m : P * (m + 1)],
                            ones_row,
                            start=False,
                            stop=True,
                        )
                    y = y_p.tile([P, T], F32, tag="y", name=f"y{l}_{m}")
                    nc.vector.scalar_tensor_tensor(
                        y, glu[m], S2, ps, AluOpType.mult, AluOpType.add
                    )
                    nc.vector.scalar_tensor_tensor(
                        u[m][:, KW - 1 :],
                        u[m][:, KW - 1 :],
                        SQ,
                        y,
                        AluOpType.mult,
                        AluOpType.add,
                    )

        # ---- final: convout (E,T) then fc_out (T,V) ----------------------
        with (
            tc.tile_pool(name="wh2e_p", bufs=1) as wh2e_p,
            tc.tile_pool(name="co_p", bufs=1) as co_p,
            tc.tile_pool(name="fcw_p", bufs=12 * kE) as fcw_p,
            tc.tile_pool(name="ot_p", bufs=6) as ot_p,
        ):
            wh2e_t = []
            for i in range(kH):
                t = wh2e_p.tile([P, E], F32R, tag=f"wh2e{i}", name=f"wh2et{i}")
                nc.sync.dma_start(t, d_wh2e[P * i : P * (i + 1), :])
                wh2e_t.append(t)
            co = []
            for m in range(kE):
                ps = pp.tile([P, T], F32, tag="ps", name=f"cops{m}")
                for k in range(kH):
                    nc.tensor.matmul(
                        ps,
                        wh2e_t[k][:, P * m : P * (m + 1)],
                        u[k][:, KW - 1 :],
                        start=(k == 0),
                        stop=(k == kH - 1 and not with_bias),
                    )
                if with_bias:
                    nc.tensor.matmul(
                        ps,
                        bh2e_t[:, P * m : P * (m + 1)],
                        ones_row,
                        start=False,
                        stop=True,
                    )
                t = co_p.tile([P, T], BF16, tag=f"co{m}", name=f"co{m}")
                nc.scalar.copy(t, ps)
                co.append(t)

            for c in range(NCH):
                fts = []
                for k in range(kE):
                    ft = fcw_p.tile([P, CH], BF16, tag="fcw", name=f"fcw{c}_{k}")
                    nc.sync.dma_start(ft, d_fcw[P * k : P * (k + 1), CH * c : CH * (c + 1)])
                    fts.append(ft)
                for m in range(mT):
                    ps = pp.tile([P, CH], F32, tag="ps", name=f"fcps{c}_{m}")
                    for k in range(kE):
                        nc.tensor.matmul(
                            ps,
                            co[k][:, P * m : P * (m + 1)],
                            fts[k],
                            start=(k == 0),
                            stop=(k == kE - 1),
                        )
                    ot = ot_p.tile([P, CH], F32, tag="ot", name=f"ot{c}_{m}")
                    nc.vector.tensor_copy(ot, ps)
                    nc.sync.dma_start(
                        d_out[P * m : P * (m + 1), CH * c : CH * (c + 1)], ot
                    )

    if legalize:
        _legalize_pe_waits(nc)
    return nc


def _host_prep(inp, T, L, KW):
    """Host-side input prep shared by kernel() and tests: embedding gather,
    transposes, conv-weight relayout."""
    f32 = np.float32
    trg = np.asarray(inp["trg"]).astype(np.int64)
    tok = np.asarray(inp["tok_emb"], dtype=f32)
    pos = np.asarray(inp["pos_emb"], dtype=f32)
    embedded = tok[trg] + pos[:T][None]  # (B,T,E)
    sq = f32(np.sqrt(np.float32(0.5)))
    embT = np.ascontiguousarray(embedded.transpose(0, 2, 1))
    embsT = np.ascontiguousarray((embedded * sq).transpose(0, 2, 1))
    encT = np.ascontiguousarray(
        np.asarray(inp["encoder_conved"], dtype=f32).transpose(0, 2, 1)
    )
    encC = np.ascontiguousarray(np.asarray(inp["encoder_combined"], dtype=f32))
    import ml_dtypes

    cw = np.ascontiguousarray(
        np.asarray(inp["conv_w"], dtype=f32).transpose(0, 3, 2, 1)
    ).astype(ml_dtypes.bfloat16)  # (L, KW, H, 2H) bf16
    return embT, embsT, encT, encC, cw


def kernel(**inputs):
    B, T, S = 8, 512, 512
    E, H, V = 512, 1024, 32000
    KW, L = 3, 6
    CH = 500

    import ml_dtypes

    f32 = np.float32
    inp = {k: np.asarray(v) for k, v in inputs.items()}
    embT, embsT, encT, encC, cw = _host_prep(inp, T, L, KW)

    dev_biases = ["emb2hid_b", "conv_b", "attn_hid2emb_b", "attn_emb2hid_b", "hid2emb_b"]
    with_bias = any(np.any(np.asarray(inp[k])) for k in dev_biases)

    nc = build_decoder_nc(
        T=T, S=S, E=E, H=H, V=V, L=L, KW=KW, CH=CH, with_bias=with_bias
    )

    base = {
        "c_pad": np.full((128, KW - 1), f32(1.0)),
        "c_ones_col": np.ones((128, 1), f32),
        "c_halfones": np.full((1, 128), f32(0.5)),
        "we2h": np.ascontiguousarray(np.asarray(inp["emb2hid_w"], dtype=f32)),
        "w1": np.ascontiguousarray(np.asarray(inp["attn_hid2emb_w"], dtype=f32)),
        "w2": np.ascontiguousarray(np.asarray(inp["attn_emb2hid_w"], dtype=f32)),
        "wh2e": np.ascontiguousarray(np.asarray(inp["hid2emb_w"], dtype=f32)),
        "fcw": np.ascontiguousarray(np.asarray(inp["fc_out_w"], dtype=f32)).astype(
            ml_dtypes.bfloat16
        ),
        "cw": cw,
    }
    if with_bias:
        base |= {
            "c_ones_row": np.ones((1, T), f32),
            "b_e2h": np.asarray(inp["emb2hid_b"], dtype=f32).reshape(1, H),
            "b1": np.asarray(inp["attn_hid2emb_b"], dtype=f32).reshape(1, E),
            "b2s2": (np.asarray(inp["attn_emb2hid_b"], dtype=f32) * f32(0.5)).reshape(1, H),
            "bh2e": np.asarray(inp["hid2emb_b"], dtype=f32).reshape(1, E),
            "cb_bf": np.ascontiguousarray(np.asarray(inp["conv_b"], dtype=f32)).astype(
                ml_dtypes.bfloat16
            ),
        }
    in_maps = [
        dict(base, embT=embT[c], embsT=embsT[c], encT=encT[c], encC=encC[c])
        for c in range(B)
    ]

    from concourse.bass_utils import run_bass_kernel_spmd

    import os

    trace = bool(os.environ.get("DECODER_TRACE"))
    res = run_bass_kernel_spmd(nc, in_maps, core_ids=list(range(B)), trace=trace)
    global _last_results
    _last_results = res
    out = np.stack([res.results[c]["out"] for c in range(B)]).astype(f32)

    fcb = np.asarray(inp["fc_out_b"], dtype=f32)
    if np.any(fcb):
        out = out + fcb[None, None, :]
    return out


# revision 49
# speedup vs baseline: 1.4230x; 1.0751x over previous
"""Trainium2 Bass kernel for the ConvS2S-style decoder (nn_Decoder).

Strategy: pure data-parallel over batch — B=8 batch elements mapped 1:1 onto
8 NeuronCores, zero cross-core communication.  Host does the (tiny) embedding
gather + a few weight transposes; each core runs the full per-batch pipeline:

    u = emb @ W_e2h                                (H,T layout, left-pad cols = 1.0)
    6 x [ conv(K=3, via 3 shifted matmuls) -> GLU
          -> attention (energy in (S,T) layout, softmax over partition dim via
             ones-matmul column sums, normalization folded in as a rank-1
             broadcast matmul) -> residual updates ]
    convout = u.T @ W_h2e ; out = convout @ W_fc   (T,V layout, 64 V-chunks)

All matmuls run as float32r (fp32 storage, fp22 multiply) which is full PE
rate for moving-dim >= 256; every tensor feeding a matmul is typed float32r
end-to-end (walrus requires matmul inputs be produced pre-rounded).
Activations stay in SBUF for the whole layer stack; only weights stream from
HBM.
"""

import numpy as np
from contextlib import ExitStack

import bass_rust
import concourse.bass as bass
import concourse.mybir as mybir
import concourse.tile as tile
from concourse.alu_op_type import AluOpType

F32 = mybir.dt.float32
F32R = mybir.dt.float32r
BF16 = mybir.dt.bfloat16
AF = mybir.ActivationFunctionType
P = 128

_last_results = None


def _legalize_pe_waits(nc):
    """Walrus packs a self-loading (fp32/fp32r) Matmult's sync waits into the
    LDWEIGHTS hw descriptor, which has a single wait slot.  Move the waits of
    any multi-wait PE compute instruction onto EventSemaphore instructions
    (one wait each) inserted just before it on the PE queue — semantically
    identical wait point, but each carrier is within the hw limit."""
    n = 0
    absorb_types = (
        "InstMatmult",
        "InstLdweights",
        "InstDMACopy",
        "InstActivation",
        "InstTensorTensor",
        "InstTensorScalarPtr",
        "InstTensorCopy",
        "InstReciprocal",
        "InstMemset",
        "InstTensorReduce",
        "InstDrain",
    )
    for fn in nc.m.functions:
        for blk in fn.blocks:
            out = []
            changed = False
            for inst in blk.instructions:
                si = inst.sync_info
                if (
                    si is not None
                    and type(inst).__name__ in absorb_types
                ):
                    waits = list(si.on_wait)
                    if len(waits) > 1:
                        for w in waits:
                            out.append(
                                mybir.InstEventSemaphore(
                                    name=f"I-pewait{n}",
                                    engine=inst.engine,
                                    sync_info=bass_rust.SyncInfo(
                                        on_wait=[w], on_update=[]
                                    ),
                                    ins=[],
                                    outs=[],
                                )
                            )
                            n += 1
                        inst.sync_info = bass_rust.SyncInfo(
                            on_wait=[], on_update=list(si.on_update)
                        )
                        changed = True
                out.append(inst)
            if changed:
                blk.instructions = out
    return n


def build_decoder_nc(T, S, E, H, V, L, KW, CH, with_bias, pad_val=1.0, legalize=True):
    """Build the per-core Bass program.  All dims must be multiples of 128
    (except V which must be a multiple of CH, CH <= 512)."""
    kE, kH, kS, mT = E // P, H // P, S // P, T // P
    NCH = V // CH
    SQ = float(np.sqrt(np.float32(0.5)))
    S2 = 0.5  # SQ**2 exactly

    nc = bass.Bass()

    d_embT = nc.declare_dram_parameter("embT", [E, T], F32R, isOutput=False)
    d_embsT = nc.declare_dram_parameter("embsT", [E, T], F32, isOutput=False)
    d_encT = nc.declare_dram_parameter("encT", [E, S], F32R, isOutput=False)
    d_encC = nc.declare_dram_parameter("encC", [S, E], F32R, isOutput=False)
    d_we2h = nc.declare_dram_parameter("we2h", [E, H], F32R, isOutput=False)
    d_w1 = nc.declare_dram_parameter("w1", [H, E], F32R, isOutput=False)
    d_w2 = nc.declare_dram_parameter("w2", [E, H], F32R, isOutput=False)
    d_wh2e = nc.declare_dram_parameter("wh2e", [H, E], F32R, isOutput=False)
    d_fcw = nc.declare_dram_parameter("fcw", [E, V], BF16, isOutput=False)
    d_cw = nc.declare_dram_parameter("cw", [L, KW, H, 2 * H], BF16, isOutput=False)
    d_cpad = nc.declare_dram_parameter("c_pad", [P, KW - 1], F32R, isOutput=False)
    d_cones = nc.declare_dram_parameter("c_ones_col", [P, 1], F32R, isOutput=False)
    d_chalf = nc.declare_dram_parameter("c_halfones", [1, P], F32R, isOutput=False)
    if with_bias:
        d_be2h = nc.declare_dram_parameter("b_e2h", [1, H], F32R, isOutput=False)
        d_b1 = nc.declare_dram_parameter("b1", [1, E], F32R, isOutput=False)
        d_b2s2 = nc.declare_dram_parameter("b2s2", [H, 1], F32, isOutput=False)
        d_bh2e = nc.declare_dram_parameter("bh2e", [1, E], F32R, isOutput=False)
    d_out = nc.declare_dram_parameter("out", [T, V], F32, isOutput=True)

    with tile.TileContext(nc) as tc, ExitStack() as ctx:
        pers = ctx.enter_context(tc.tile_pool(name="pers", bufs=1))
        pp = ctx.enter_context(tc.tile_pool(name="pp", bufs=8, space="PSUM"))

        # ---- persistent SBUF tensors -------------------------------------
        u = []
        for i in range(kH):
            t = pers.tile([P, T + KW - 1], F32R, tag=f"u{i}", name=f"u{i}")
            u.append(t)
            nc.sync.dma_start(t[:, 0 : KW - 1], d_cpad[:, :])
        embs_t = []
        for i in range(kE):
            t = pers.tile([P, T], F32, tag=f"embs{i}", name=f"embs{i}")
            nc.sync.dma_start(t, d_embsT[P * i : P * (i + 1), :])
            embs_t.append(t)
        encT_t = []
        for i in range(kE):
            t = pers.tile([P, S], F32R, tag=f"encT{i}", name=f"encTt{i}")
            nc.sync.dma_start(t, d_encT[P * i : P * (i + 1), :])
            encT_t.append(t)
        encC_t = []
        for i in range(kS):
            t = pers.tile([P, E], F32R, tag=f"encC{i}", name=f"encCt{i}")
            nc.sync.dma_start(t, d_encC[P * i : P * (i + 1), :])
            encC_t.append(t)
        w1_t = []
        for i in range(kH):
            t = pers.tile([P, E], F32R, tag=f"w1_{i}", name=f"w1t{i}")
            nc.sync.dma_start(t, d_w1[P * i : P * (i + 1), :])
            w1_t.append(t)
        w2_t = []
        for i in range(kE):
            t = pers.tile([P, H], F32R, tag=f"w2_{i}", name=f"w2t{i}")
            nc.sync.dma_start(t, d_w2[P * i : P * (i + 1), :])
            w2_t.append(t)
        ones_col = pers.tile([P, 1], F32R, tag="ones_col", name="ones_col")
        nc.sync.dma_start(ones_col, d_cones[:, :])
        halfones = pers.tile([1, P], F32R, tag="halfones", name="halfones")
        nc.sync.dma_start(halfones, d_chalf[:, :])

        if with_bias:
            d_crow = nc.declare_dram_parameter("c_ones_row", [1, T], F32R, isOutput=False)
            ones_row = pers.tile([1, T], F32R, tag="ones_row", name="ones_row")
            nc.sync.dma_start(ones_row, d_crow[:, :])
            ones_row_bf = pers.tile([1, T], BF16, tag="ones_row_bf", name="ones_row_bf")
            nc.vector.tensor_copy(ones_row_bf, ones_row)
            be2h_t = pers.tile([1, H], F32R, tag="be2h", name="be2h_t")
            nc.sync.dma_start(be2h_t, d_be2h[:, :])
            b1_t = pers.tile([1, E], F32R, tag="b1", name="b1_t")
            nc.sync.dma_start(b1_t, d_b1[:, :])
            b2s2_sb = []
            for m in range(kH):
                t = pers.tile([P, 1], F32, tag=f"b2s2_{m}", name=f"b2s2_{m}")
                nc.sync.dma_start(t, d_b2s2[P * m : P * (m + 1), :])
                b2s2_sb.append(t)
            bh2e_t = pers.tile([1, E], F32R, tag="bh2e", name="bh2e_t")
            nc.sync.dma_start(bh2e_t, d_bh2e[:, :])
            d_cbf = nc.declare_dram_parameter("cb_bf", [L, 2 * H], BF16, isOutput=False)
            cb_t = []
            for l in range(L):
                t = pers.tile([1, 2 * H], BF16, tag=f"cb{l}", name=f"cb_t{l}")
                nc.sync.dma_start(t, d_cbf[l : l + 1, :])
                cb_t.append(t)

        # ---- init: u[:, KW-1:] = W_e2h.T @ embT (+ b_e2h) ----------------
        ubf_pers = ctx.enter_context(tc.tile_pool(name="ubf_p", bufs=kH))
        ubf = []
        with tc.tile_pool(name="initp", bufs=1) as initp:
            embT_t = []
            for i in range(kE):
                t = initp.tile([P, T], F32R, tag=f"embT{i}", name=f"embTt{i}")
                nc.sync.dma_start(t, d_embT[P * i : P * (i + 1), :])
                embT_t.append(t)
            we2h_t = []
            for i in range(kE):
                t = initp.tile([P, H], F32R, tag=f"we2h{i}", name=f"we2ht{i}")
                nc.sync.dma_start(t, d_we2h[P * i : P * (i + 1), :])
                we2h_t.append(t)
            for m in range(kH):
                ps = pp.tile([P, T], F32, tag="ps", name=f"initps{m}")
                for k in range(kE):
                    nc.tensor.matmul(
                        ps,
                        we2h_t[k][:, P * m : P * (m + 1)],
                        embT_t[k],
                        start=(k == 0),
                        stop=(k == kE - 1 and not with_bias),
                    )
                if with_bias:
                    nc.tensor.matmul(
                        ps,
                        be2h_t[:, P * m : P * (m + 1)],
                        ones_row,
                        start=False,
                        stop=True,
                    )
                nc.scalar.copy(u[m][:, KW - 1 :], ps)
                t = ubf_pers.tile([P, T + KW - 1], BF16, tag="ubf", name=f"ubf0_{m}")
                nc.scalar.copy(t, u[m])
                ubf.append(t)

        # ---- layer stack -------------------------------------------------
        n_stripes = KW * kH
        with (
            tc.tile_pool(name="wconv_p", bufs=n_stripes + 8) as wconv_p,
            tc.tile_pool(name="sig_p", bufs=kH) as sig_p,
            tc.tile_pool(name="glu_p", bufs=kH) as glu_p,
            tc.tile_pool(name="comb_p", bufs=kE) as comb_p,
            tc.tile_pool(name="ex_p", bufs=kS) as ex_p,
            tc.tile_pool(name="att_p", bufs=kE) as att_p,
            tc.tile_pool(name="rec_p", bufs=1) as rec_p,
            tc.tile_pool(name="y_p", bufs=2) as y_p,
        ):
            for l in range(L):
                u_bf = ubf
                # conv + GLU: g-half (gate) first, then a-half; m-outer with
                # the half's full weight set resident so each psum finishes
                # early and GLU/attention overlap the remaining conv matmuls.
                sig = []
                glu = []
                for half in (1, 0):  # 1 = gate channels [H:2H), 0 = a [0:H)
                    wsts = []
                    for kw in range(KW):
                        for k in range(kH):
                            wst = wconv_p.tile(
                                [P, H], BF16, tag="wst", name=f"wst{l}_{half}_{kw}_{k}"
                            )
                            nc.sync.dma_start(
                                wst,
                                d_cw[l, kw, P * k : P * (k + 1), H * half : H * (half + 1)],
                            )
                            wsts.append((kw, k, wst))
                    for m in range(kH):
                        cps = pp.tile([P, T], F32, tag="ps", name=f"cps{l}_{half}_{m}")
                        for i_mm, (kw, k, wst) in enumerate(wsts):
                            nc.tensor.matmul(
                                cps,
                                wst[:, P * m : P * (m + 1)],
                                u_bf[k][:, kw : kw + T],
                                start=(i_mm == 0),
                                stop=(i_mm == n_stripes - 1 and not with_bias),
                            )
                        if with_bias:
                            nc.tensor.matmul(
                                cps,
                                cb_t[l][
                                    :, half * H + P * m : half * H + P * (m + 1)
                                ],
                                ones_row_bf,
                                start=False,
                                stop=True,
                            )
                        if half == 1:
                            sg = sig_p.tile([P, T], BF16, tag="sig", name=f"sig{l}_{m}")
                            nc.scalar.activation(sg, cps, AF.Sigmoid)
                            sig.append(sg)
                        else:
                            g = glu_p.tile([P, T], F32R, tag="glu", name=f"glu{l}_{m}")
                            nc.vector.tensor_mul(g, cps, sig[m])
                            glu.append(g)

                # attention: combined = (glu.T @ w1 (+b1)) * s + emb*s, (E,T)
                comb = []
                for m in range(kE):
                    ps = pp.tile([P, T], F32, tag="ps", name=f"ceps{l}_{m}")
                    for k in range(kH):
                        nc.tensor.matmul(
                            ps,
                            w1_t[k][:, P * m : P * (m + 1)],
                            glu[k],
                            start=(k == 0),
                            stop=(k == kH - 1 and not with_bias),
                        )
                    if with_bias:
                        nc.tensor.matmul(
                            ps,
                            b1_t[:, P * m : P * (m + 1)],
                            ones_row,
                            start=False,
                            stop=True,
                        )
                    c = comb_p.tile([P, T], F32R, tag="comb", name=f"comb{l}_{m}")
                    nc.vector.scalar_tensor_tensor(
                        c, ps, SQ, embs_t[m], AluOpType.mult, AluOpType.add
                    )
                    comb.append(c)

                # energy in (S, T) layout; exp elementwise (energies are
                # bounded ~|22| for this model, fp32-safe without max-sub)
                ex = []
                for m in range(kS):
                    ps = pp.tile([P, T], F32, tag="ps", name=f"enps{l}_{m}")
                    for k in range(kE):
                        nc.tensor.matmul(
                            ps,
                            encT_t[k][:, P * m : P * (m + 1)],
                            comb[k],
                            start=(k == 0),
                            stop=(k == kE - 1),
                        )
                    e = ex_p.tile([P, T], F32R, tag="ex", name=f"ex{l}_{m}")
                    nc.scalar.activation(e, ps, AF.Exp)
                    ex.append(e)

                # column sums over S (partition dim) via ones matmul; then
                # rec_bc[p, t] = 0.5 / sums[t] broadcast to 128 partitions
                sps = pp.tile([1, T], F32, tag="ps", name=f"sums{l}")
                for k in range(kS):
                    nc.tensor.matmul(
                        sps, ones_col, ex[k], start=(k == 0), stop=(k == kS - 1)
                    )
                rec = rec_p.tile([1, T], F32R, tag="rec", name=f"rec{l}")
                with nc.allow_low_precision(reason="softmax recip feeds f32r matmul"):
                    nc.vector.reciprocal(rec, sps)
                bps = pp.tile([P, T], F32, tag="ps", name=f"bps{l}")
                nc.tensor.matmul(bps, halfones, rec, start=True, stop=True)
                rbc = rec_p.tile([P, T], F32, tag="rbc", name=f"rbc{l}")
                nc.scalar.copy(rbc, bps)

                # attended (E,T), unnormalized — normalization (x rbc) is
                # applied after the att2 matmul so the reciprocal chain
                # overlaps PE work instead of stalling it
                att = []
                for m in range(kE):
                    ps = pp.tile([P, T], F32, tag="ps", name=f"atps{l}_{m}")
                    for k in range(kS):
                        nc.tensor.matmul(
                            ps,
                            encC_t[k][:, P * m : P * (m + 1)],
                            ex[k],
                            start=(k == 0),
                            stop=(k == kS - 1),
                        )
                    a = att_p.tile([P, T], F32R, tag="att", name=f"att{l}_{m}")
                    nc.scalar.copy(a, ps)
                    att.append(a)

                # att2 = w2.T @ att (+ b2*s^2/rbc); then per m-tile:
                #   x1 = att2_psum * rbc          (DVE, psum operand)
                #   y  = glu*s^2 + x1             (GPSIMD, sbuf only)
                #   u  = u*s + y                  (DVE)
                #   ubf= bf16(u)                  (ACT) -> next layer's conv
                next_ubf = []
                for m in range(kH):
                    ps = pp.tile([P, T], F32, tag="ps", name=f"a2ps{l}_{m}")
                    for k in range(kE):
                        nc.tensor.matmul(
                            ps,
                            w2_t[k][:, P * m : P * (m + 1)],
                            att[k],
                            start=(k == 0),
                            stop=(k == kE - 1),
                        )
                    x1 = y_p.tile([P, T], F32, tag="x1", name=f"x1_{l}_{m}")
                    nc.vector.tensor_mul(x1, ps, rbc)
                    if with_bias:
                        nc.vector.tensor_scalar_add(
                            x1, x1, b2s2_sb[m]
                        )
                    y = y_p.tile([P, T], F32, tag="y", name=f"y{l}_{m}")
                    nc.vector.scalar_tensor_tensor(
                        y, glu[m], S2, x1, AluOpType.mult, AluOpType.add
                    )
                    nc.vector.scalar_tensor_tensor(
                        u[m][:, KW - 1 :],
                        u[m][:, KW - 1 :],
                        SQ,
                        y,
                        AluOpType.mult,
                        AluOpType.add,
                    )
                    nb = ubf_pers.tile(
                        [P, T + KW - 1], BF16, tag="ubf", name=f"ubf{l + 1}_{m}"
                    )
                    nc.gpsimd.tensor_copy(nb, u[m])
                    next_ubf.append(nb)
                ubf = next_ubf

        # ---- final: convout (E,T) then fc_out (T,V) ----------------------
        with (
            tc.tile_pool(name="wh2e_p", bufs=1) as wh2e_p,
            tc.tile_pool(name="co_p", bufs=1) as co_p,
            tc.tile_pool(name="fcw_p", bufs=3 * kE) as fcw_p,
            tc.tile_pool(name="ot_p", bufs=mT + 2) as ot_p,
        ):
            wh2e_t = []
            for i in range(kH):
                t = wh2e_p.tile([P, E], F32R, tag=f"wh2e{i}", name=f"wh2et{i}")
                nc.sync.dma_start(t, d_wh2e[P * i : P * (i + 1), :])
                wh2e_t.append(t)
            co = []
            for m in range(kE):
                ps = pp.tile([P, T], F32, tag="ps", name=f"cops{m}")
                for k in range(kH):
                    nc.tensor.matmul(
                        ps,
                        wh2e_t[k][:, P * m : P * (m + 1)],
                        u[k][:, KW - 1 :],
                        start=(k == 0),
                        stop=(k == kH - 1 and not with_bias),
                    )
                if with_bias:
                    nc.tensor.matmul(
                        ps,
                        bh2e_t[:, P * m : P * (m + 1)],
                        ones_row,
                        start=False,
                        stop=True,
                    )
                t = co_p.tile([P, T], BF16, tag=f"co{m}", name=f"co{m}")
                nc.scalar.copy(t, ps)
                co.append(t)

            # chunk groups of GS: bigger DMA transfers for fcw reads and
            # output writes (4x inner-contig), psum stays one CH-chunk
            GS = 4 if NCH % 4 == 0 else (2 if NCH % 2 == 0 else 1)
            GW = GS * CH
            for cg in range(NCH // GS):
                fts = []
                for k in range(kE):
                    ft = fcw_p.tile([P, GW], BF16, tag="fcw", name=f"fcw{cg}_{k}")
                    nc.sync.dma_start(
                        ft, d_fcw[P * k : P * (k + 1), GW * cg : GW * (cg + 1)]
                    )
                    fts.append(ft)
                for m in range(mT):
                    ot = ot_p.tile([P, GW], F32, tag="ot", name=f"ot{cg}_{m}")
                    for sub in range(GS):
                        ps = pp.tile([P, CH], F32, tag="ps", name=f"fcps{cg}_{m}_{sub}")
                        for k in range(kE):
                            nc.tensor.matmul(
                                ps,
                                co[k][:, P * m : P * (m + 1)],
                                fts[k][:, CH * sub : CH * (sub + 1)],
                                start=(k == 0),
                                stop=(k == kE - 1),
                            )
                        nc.vector.tensor_copy(ot[:, CH * sub : CH * (sub + 1)], ps)
                    nc.sync.dma_start(
                        d_out[P * m : P * (m + 1), GW * cg : GW * (cg + 1)], ot
                    )

    if legalize:
        _legalize_pe_waits(nc)
    return nc


def _host_prep(inp, T, L, KW):
    """Host-side input prep shared by kernel() and tests: embedding gather,
    transposes, conv-weight relayout."""
    f32 = np.float32
    trg = np.asarray(inp["trg"]).astype(np.int64)
    tok = np.asarray(inp["tok_emb"], dtype=f32)
    pos = np.asarray(inp["pos_emb"], dtype=f32)
    embedded = tok[trg] + pos[:T][None]  # (B,T,E)
    sq = f32(np.sqrt(np.float32(0.5)))
    embT = np.ascontiguousarray(embedded.transpose(0, 2, 1))
    embsT = np.ascontiguousarray((embedded * sq).transpose(0, 2, 1))
    encT = np.ascontiguousarray(
        np.asarray(inp["encoder_conved"], dtype=f32).transpose(0, 2, 1)
    )
    encC = np.ascontiguousarray(np.asarray(inp["encoder_combined"], dtype=f32))
    import ml_dtypes

    cw = np.ascontiguousarray(
        np.asarray(inp["conv_w"], dtype=f32).transpose(0, 3, 2, 1)
    ).astype(ml_dtypes.bfloat16)  # (L, KW, H, 2H) bf16
    return embT, embsT, encT, encC, cw


def kernel(**inputs):
    B, T, S = 8, 512, 512
    E, H, V = 512, 1024, 32000
    KW, L = 3, 6
    CH = 500

    import ml_dtypes

    f32 = np.float32
    inp = {k: np.asarray(v) for k, v in inputs.items()}
    embT, embsT, encT, encC, cw = _host_prep(inp, T, L, KW)

    dev_biases = ["emb2hid_b", "conv_b", "attn_hid2emb_b", "attn_emb2hid_b", "hid2emb_b"]
    with_bias = any(np.any(np.asarray(inp[k])) for k in dev_biases)

    nc = build_decoder_nc(
        T=T, S=S, E=E, H=H, V=V, L=L, KW=KW, CH=CH, with_bias=with_bias
    )

    base = {
        "c_pad": np.full((128, KW - 1), f32(1.0)),
        "c_ones_col": np.ones((128, 1), f32),
        "c_halfones": np.full((1, 128), f32(0.5)),
        "we2h": np.ascontiguousarray(np.asarray(inp["emb2hid_w"], dtype=f32)),
        "w1": np.ascontiguousarray(np.asarray(inp["attn_hid2emb_w"], dtype=f32)),
        "w2": np.ascontiguousarray(np.asarray(inp["attn_emb2hid_w"], dtype=f32)),
        "wh2e": np.ascontiguousarray(np.asarray(inp["hid2emb_w"], dtype=f32)),
        "fcw": np.ascontiguousarray(np.asarray(inp["fc_out_w"], dtype=f32)).astype(
            ml_dtypes.bfloat16
        ),
        "cw": cw,
    }
    if with_bias:
        base |= {
            "c_ones_row": np.ones((1, T), f32),
            "b_e2h": np.asarray(inp["emb2hid_b"], dtype=f32).reshape(1, H),
            "b1": np.asarray(inp["attn_hid2emb_b"], dtype=f32).reshape(1, E),
            "b2s2": (np.asarray(inp["attn_emb2hid_b"], dtype=f32) * f32(0.5)).reshape(H, 1),
            "bh2e": np.asarray(inp["hid2emb_b"], dtype=f32).reshape(1, E),
            "cb_bf": np.ascontiguousarray(np.asarray(inp["conv_b"], dtype=f32)).astype(
                ml_dtypes.bfloat16
            ),
        }
    in_maps = [
        dict(base, embT=embT[c], embsT=embsT[c], encT=encT[c], encC=encC[c])
        for c in range(B)
    ]

    from concourse.bass_utils import run_bass_kernel_spmd

    import os

    trace = bool(os.environ.get("DECODER_TRACE"))
    res = run_bass_kernel_spmd(nc, in_maps, core_ids=list(range(B)), trace=trace)
    global _last_results
    _last_results = res
    out = np.stack([res.results[c]["out"] for c in range(B)]).astype(f32)

    fcb = np.asarray(inp["fc_out_b"], dtype=f32)
    if np.any(fcb):
        out = out + fcb[None, None, :]
    return out


# revision 50
# speedup vs baseline: 1.4699x; 1.0329x over previous
"""Trainium2 Bass kernel for the ConvS2S-style decoder (nn_Decoder).

Strategy: pure data-parallel over batch — B=8 batch elements mapped 1:1 onto
8 NeuronCores, zero cross-core communication.  Host does the (tiny) embedding
gather + a few weight transposes; each core runs the full per-batch pipeline:

    u = emb @ W_e2h                                (H,T layout, left-pad cols = 1.0)
    6 x [ conv(K=3, via 3 shifted matmuls) -> GLU
          -> attention (energy in (S,T) layout, softmax over partition dim via
             ones-matmul column sums, normalization folded in as a rank-1
             broadcast matmul) -> residual updates ]
    convout = u.T @ W_h2e ; out = convout @ W_fc   (T,V layout, 64 V-chunks)

All matmuls run as float32r (fp32 storage, fp22 multiply) which is full PE
rate for moving-dim >= 256; every tensor feeding a matmul is typed float32r
end-to-end (walrus requires matmul inputs be produced pre-rounded).
Activations stay in SBUF for the whole layer stack; only weights stream from
HBM.
"""

import numpy as np
from contextlib import ExitStack

import bass_rust
import concourse.bass as bass
import concourse.mybir as mybir
import concourse.tile as tile
from concourse.alu_op_type import AluOpType

F32 = mybir.dt.float32
F32R = mybir.dt.float32r
BF16 = mybir.dt.bfloat16
AF = mybir.ActivationFunctionType
P = 128

_last_results = None


def _legalize_pe_waits(nc):
    """Walrus packs a self-loading (fp32/fp32r) Matmult's sync waits into the
    LDWEIGHTS hw descriptor, which has a single wait slot.  Move the waits of
    any multi-wait PE compute instruction onto EventSemaphore instructions
    (one wait each) inserted just before it on the PE queue — semantically
    identical wait point, but each carrier is within the hw limit."""
    n = 0
    absorb_types = (
        "InstMatmult",
        "InstLdweights",
        "InstDMACopy",
        "InstActivation",
        "InstTensorTensor",
        "InstTensorScalarPtr",
        "InstTensorCopy",
        "InstReciprocal",
        "InstMemset",
        "InstTensorReduce",
        "InstDrain",
    )
    for fn in nc.m.functions:
        for blk in fn.blocks:
            out = []
            changed = False
            for inst in blk.instructions:
                si = inst.sync_info
                if (
                    si is not None
                    and type(inst).__name__ in absorb_types
                ):
                    waits = list(si.on_wait)
                    if len(waits) > 1:
                        for w in waits:
                            out.append(
                                mybir.InstEventSemaphore(
                                    name=f"I-pewait{n}",
                                    engine=inst.engine,
                                    sync_info=bass_rust.SyncInfo(
                                        on_wait=[w], on_update=[]
                                    ),
                                    ins=[],
                                    outs=[],
                                )
                            )
                            n += 1
                        inst.sync_info = bass_rust.SyncInfo(
                            on_wait=[], on_update=list(si.on_update)
                        )
                        changed = True
                out.append(inst)
            if changed:
                blk.instructions = out
    return n


def build_decoder_nc(T, S, E, H, V, L, KW, CH, with_bias, pad_val=1.0, legalize=True):
    """Build the per-core Bass program.  All dims must be multiples of 128
    (except V which must be a multiple of CH, CH <= 512)."""
    kE, kH, kS, mT = E // P, H // P, S // P, T // P
    NCH = V // CH
    SQ = float(np.sqrt(np.float32(0.5)))
    S2 = 0.5  # SQ**2 exactly

    nc = bass.Bass()

    d_embT = nc.declare_dram_parameter("embT", [E, T], F32R, isOutput=False)
    d_embsT = nc.declare_dram_parameter("embsT", [E, T], F32, isOutput=False)
    d_encT = nc.declare_dram_parameter("encT", [E, S], F32R, isOutput=False)
    d_encC = nc.declare_dram_parameter("encC", [S, E], F32R, isOutput=False)
    d_we2h = nc.declare_dram_parameter("we2h", [E, H], F32R, isOutput=False)
    d_w1 = nc.declare_dram_parameter("w1", [H, E], F32R, isOutput=False)
    d_w2 = nc.declare_dram_parameter("w2", [E, H], F32R, isOutput=False)
    d_wh2e = nc.declare_dram_parameter("wh2e", [H, E], F32R, isOutput=False)
    d_fcw = nc.declare_dram_parameter("fcw", [E, V], BF16, isOutput=False)
    d_cw = nc.declare_dram_parameter("cw", [L, KW, H, 2 * H], BF16, isOutput=False)
    d_cpad = nc.declare_dram_parameter("c_pad", [P, KW - 1], F32R, isOutput=False)
    d_cones = nc.declare_dram_parameter("c_ones_col", [P, 1], F32R, isOutput=False)
    d_chalf = nc.declare_dram_parameter("c_halfones", [1, P], F32R, isOutput=False)
    if with_bias:
        d_be2h = nc.declare_dram_parameter("b_e2h", [1, H], F32R, isOutput=False)
        d_b1 = nc.declare_dram_parameter("b1", [1, E], F32R, isOutput=False)
        d_b2s2 = nc.declare_dram_parameter("b2s2", [H, 1], F32, isOutput=False)
        d_bh2e = nc.declare_dram_parameter("bh2e", [1, E], F32R, isOutput=False)
    d_out = nc.declare_dram_parameter("out", [T, V], F32, isOutput=True)

    with tile.TileContext(nc) as tc, ExitStack() as ctx:
        pers = ctx.enter_context(tc.tile_pool(name="pers", bufs=1))
        pp = ctx.enter_context(tc.tile_pool(name="pp", bufs=8, space="PSUM"))

        # ---- persistent SBUF tensors -------------------------------------
        u = []
        for i in range(kH):
            t = pers.tile([P, T + KW - 1], F32R, tag=f"u{i}", name=f"u{i}")
            u.append(t)
            nc.sync.dma_start(t[:, 0 : KW - 1], d_cpad[:, :])
        embs_t = []
        for i in range(kE):
            t = pers.tile([P, T], F32, tag=f"embs{i}", name=f"embs{i}")
            nc.sync.dma_start(t, d_embsT[P * i : P * (i + 1), :])
            embs_t.append(t)
        encT_t = []
        for i in range(kE):
            t = pers.tile([P, S], F32R, tag=f"encT{i}", name=f"encTt{i}")
            nc.sync.dma_start(t, d_encT[P * i : P * (i + 1), :])
            encT_t.append(t)
        encC_t = []
        for i in range(kS):
            t = pers.tile([P, E], F32R, tag=f"encC{i}", name=f"encCt{i}")
            nc.sync.dma_start(t, d_encC[P * i : P * (i + 1), :])
            encC_t.append(t)
        w1_t = []
        for i in range(kH):
            t = pers.tile([P, E], F32R, tag=f"w1_{i}", name=f"w1t{i}")
            nc.sync.dma_start(t, d_w1[P * i : P * (i + 1), :])
            w1_t.append(t)
        w2_t = []
        for i in range(kE):
            t = pers.tile([P, H], F32R, tag=f"w2_{i}", name=f"w2t{i}")
            nc.sync.dma_start(t, d_w2[P * i : P * (i + 1), :])
            w2_t.append(t)
        ones_col = pers.tile([P, 1], F32R, tag="ones_col", name="ones_col")
        nc.sync.dma_start(ones_col, d_cones[:, :])
        halfones = pers.tile([1, P], F32R, tag="halfones", name="halfones")
        nc.sync.dma_start(halfones, d_chalf[:, :])

        if with_bias:
            d_crow = nc.declare_dram_parameter("c_ones_row", [1, T], F32R, isOutput=False)
            ones_row = pers.tile([1, T], F32R, tag="ones_row", name="ones_row")
            nc.sync.dma_start(ones_row, d_crow[:, :])
            ones_row_bf = pers.tile([1, T], BF16, tag="ones_row_bf", name="ones_row_bf")
            nc.vector.tensor_copy(ones_row_bf, ones_row)
            be2h_t = pers.tile([1, H], F32R, tag="be2h", name="be2h_t")
            nc.sync.dma_start(be2h_t, d_be2h[:, :])
            b1_t = pers.tile([1, E], F32R, tag="b1", name="b1_t")
            nc.sync.dma_start(b1_t, d_b1[:, :])
            b2s2_sb = []
            for m in range(kH):
                t = pers.tile([P, 1], F32, tag=f"b2s2_{m}", name=f"b2s2_{m}")
                nc.sync.dma_start(t, d_b2s2[P * m : P * (m + 1), :])
                b2s2_sb.append(t)
            bh2e_t = pers.tile([1, E], F32R, tag="bh2e", name="bh2e_t")
            nc.sync.dma_start(bh2e_t, d_bh2e[:, :])
            d_cbf = nc.declare_dram_parameter("cb_bf", [L, 2 * H], BF16, isOutput=False)
            cb_t = []
            for l in range(L):
                t = pers.tile([1, 2 * H], BF16, tag=f"cb{l}", name=f"cb_t{l}")
                nc.sync.dma_start(t, d_cbf[l : l + 1, :])
                cb_t.append(t)

        # ---- init: u[:, KW-1:] = W_e2h.T @ embT (+ b_e2h) ----------------
        ubf_pers = ctx.enter_context(tc.tile_pool(name="ubf_p", bufs=kH))
        ubf = []
        with tc.tile_pool(name="initp", bufs=1) as initp:
            embT_t = []
            for i in range(kE):
                t = initp.tile([P, T], F32R, tag=f"embT{i}", name=f"embTt{i}")
                nc.sync.dma_start(t, d_embT[P * i : P * (i + 1), :])
                embT_t.append(t)
            we2h_t = []
            for i in range(kE):
                t = initp.tile([P, H], F32R, tag=f"we2h{i}", name=f"we2ht{i}")
                nc.sync.dma_start(t, d_we2h[P * i : P * (i + 1), :])
                we2h_t.append(t)
            for m in range(kH):
                ps = pp.tile([P, T], F32, tag="ps", name=f"initps{m}")
                for k in range(kE):
                    nc.tensor.matmul(
                        ps,
                        we2h_t[k][:, P * m : P * (m + 1)],
                        embT_t[k],
                        start=(k == 0),
                        stop=(k == kE - 1 and not with_bias),
                    )
                if with_bias:
                    nc.tensor.matmul(
                        ps,
                        be2h_t[:, P * m : P * (m + 1)],
                        ones_row,
                        start=False,
                        stop=True,
                    )
                nc.scalar.copy(u[m][:, KW - 1 :], ps)
                t = ubf_pers.tile([P, T + KW - 1], BF16, tag="ubf", name=f"ubf0_{m}")
                nc.scalar.copy(t, u[m])
                ubf.append(t)

        # ---- layer stack -------------------------------------------------
        n_stripes = KW * kH
        with (
            tc.tile_pool(name="wconv_p", bufs=n_stripes + 8) as wconv_p,
            tc.tile_pool(name="sig_p", bufs=kH) as sig_p,
            tc.tile_pool(name="glu_p", bufs=kH) as glu_p,
            tc.tile_pool(name="comb_p", bufs=kE) as comb_p,
            tc.tile_pool(name="ex_p", bufs=kS) as ex_p,
            tc.tile_pool(name="att_p", bufs=kE) as att_p,
            tc.tile_pool(name="rec_p", bufs=1) as rec_p,
            tc.tile_pool(name="y_p", bufs=2) as y_p,
        ):
            for l in range(L):
                u_bf = ubf
                # conv + GLU: g-half (gate) first, then a-half; m-outer with
                # the half's full weight set resident so each psum finishes
                # early and GLU/attention overlap the remaining conv matmuls.
                sig = []
                glu = []
                for half in (1, 0):  # 1 = gate channels [H:2H), 0 = a [0:H)
                    wsts = []
                    for kw in range(KW):
                        for k in range(kH):
                            wst = wconv_p.tile(
                                [P, H], BF16, tag="wst", name=f"wst{l}_{half}_{kw}_{k}"
                            )
                            nc.sync.dma_start(
                                wst,
                                d_cw[l, kw, P * k : P * (k + 1), H * half : H * (half + 1)],
                            )
                            wsts.append((kw, k, wst))
                    for m in range(kH):
                        cps = pp.tile([P, T], F32, tag="ps", name=f"cps{l}_{half}_{m}")
                        for i_mm, (kw, k, wst) in enumerate(wsts):
                            nc.tensor.matmul(
                                cps,
                                wst[:, P * m : P * (m + 1)],
                                u_bf[k][:, kw : kw + T],
                                start=(i_mm == 0),
                                stop=(i_mm == n_stripes - 1 and not with_bias),
                            )
                        if with_bias:
                            nc.tensor.matmul(
                                cps,
                                cb_t[l][
                                    :, half * H + P * m : half * H + P * (m + 1)
                                ],
                                ones_row_bf,
                                start=False,
                                stop=True,
                            )
                        if half == 1:
                            sg = sig_p.tile([P, T], BF16, tag="sig", name=f"sig{l}_{m}")
                            nc.scalar.activation(sg, cps, AF.Sigmoid)
                            sig.append(sg)
                        else:
                            g = glu_p.tile([P, T], F32R, tag="glu", name=f"glu{l}_{m}")
                            nc.vector.tensor_mul(g, cps, sig[m])
                            glu.append(g)

                # attention: combined = (glu.T @ w1 (+b1)) * s + emb*s, (E,T)
                comb = []
                for m in range(kE):
                    ps = pp.tile([P, T], F32, tag="ps", name=f"ceps{l}_{m}")
                    for k in range(kH):
                        nc.tensor.matmul(
                            ps,
                            w1_t[k][:, P * m : P * (m + 1)],
                            glu[k],
                            start=(k == 0),
                            stop=(k == kH - 1 and not with_bias),
                        )
                    if with_bias:
                        nc.tensor.matmul(
                            ps,
                            b1_t[:, P * m : P * (m + 1)],
                            ones_row,
                            start=False,
                            stop=True,
                        )
                    c = comb_p.tile([P, T], F32R, tag="comb", name=f"comb{l}_{m}")
                    nc.vector.scalar_tensor_tensor(
                        c, ps, SQ, embs_t[m], AluOpType.mult, AluOpType.add
                    )
                    comb.append(c)

                # energy in (S, T) layout; exp elementwise (energies are
                # bounded ~|22| for this model, fp32-safe without max-sub)
                ex = []
                for m in range(kS):
                    ps = pp.tile([P, T], F32, tag="ps", name=f"enps{l}_{m}")
                    for k in range(kE):
                        nc.tensor.matmul(
                            ps,
                            encT_t[k][:, P * m : P * (m + 1)],
                            comb[k],
                            start=(k == 0),
                            stop=(k == kE - 1),
                        )
                    e = ex_p.tile([P, T], F32R, tag="ex", name=f"ex{l}_{m}")
                    nc.scalar.activation(e, ps, AF.Exp)
                    ex.append(e)

                # column sums over S (partition dim) via ones matmul; then
                # rec_bc[p, t] = 0.5 / sums[t] broadcast to 128 partitions
                sps = pp.tile([1, T], F32, tag="ps", name=f"sums{l}")
                for k in range(kS):
                    nc.tensor.matmul(
                        sps, ones_col, ex[k], start=(k == 0), stop=(k == kS - 1)
                    )
                rec = rec_p.tile([1, T], F32R, tag="rec", name=f"rec{l}")
                with nc.allow_low_precision(reason="softmax recip feeds f32r matmul"):
                    nc.vector.reciprocal(rec, sps)
                bps = pp.tile([P, T], F32, tag="ps", name=f"bps{l}")
                nc.tensor.matmul(bps, halfones, rec, start=True, stop=True)
                rbc = rec_p.tile([P, T], F32, tag="rbc", name=f"rbc{l}")
                nc.scalar.copy(rbc, bps)

                # attended (E,T), unnormalized — normalization (x rbc) is
                # applied after the att2 matmul so the reciprocal chain
                # overlaps PE work instead of stalling it
                att = []
                for m in range(kE):
                    ps = pp.tile([P, T], F32, tag="ps", name=f"atps{l}_{m}")
                    for k in range(kS):
                        nc.tensor.matmul(
                            ps,
                            encC_t[k][:, P * m : P * (m + 1)],
                            ex[k],
                            start=(k == 0),
                            stop=(k == kS - 1),
                        )
                    a = att_p.tile([P, T], F32R, tag="att", name=f"att{l}_{m}")
                    nc.scalar.copy(a, ps)
                    att.append(a)

                # att2 = w2.T @ att (+ b2*s^2/rbc); then per m-tile:
                #   x1 = att2_psum * rbc          (DVE, psum operand)
                #   y  = glu*s^2 + x1             (GPSIMD, sbuf only)
                #   u  = u*s + y                  (DVE)
                #   ubf= bf16(u)                  (ACT) -> next layer's conv
                next_ubf = []
                for m in range(kH):
                    ps = pp.tile([P, T], F32, tag="ps", name=f"a2ps{l}_{m}")
                    for k in range(kE):
                        nc.tensor.matmul(
                            ps,
                            w2_t[k][:, P * m : P * (m + 1)],
                            att[k],
                            start=(k == 0),
                            stop=(k == kE - 1),
                        )
                    x1 = y_p.tile([P, T], F32, tag="x1", name=f"x1_{l}_{m}")
                    nc.vector.tensor_mul(x1, ps, rbc)
                    if with_bias:
                        nc.vector.tensor_scalar_add(
                            x1, x1, b2s2_sb[m]
                        )
                    y = y_p.tile([P, T], F32, tag="y", name=f"y{l}_{m}")
                    nc.vector.scalar_tensor_tensor(
                        y, glu[m], S2, x1, AluOpType.mult, AluOpType.add
                    )
                    nc.vector.scalar_tensor_tensor(
                        u[m][:, KW - 1 :],
                        u[m][:, KW - 1 :],
                        SQ,
                        y,
                        AluOpType.mult,
                        AluOpType.add,
                    )
                    nb = ubf_pers.tile(
                        [P, T + KW - 1], BF16, tag="ubf", name=f"ubf{l + 1}_{m}"
                    )
                    nc.scalar.copy(nb, u[m])
                    next_ubf.append(nb)
                ubf = next_ubf

        # ---- final: convout (E,T) then fc_out (T,V) ----------------------
        with (
            tc.tile_pool(name="wh2e_p", bufs=1) as wh2e_p,
            tc.tile_pool(name="co_p", bufs=1) as co_p,
            tc.tile_pool(name="fcw_p", bufs=3 * kE) as fcw_p,
            tc.tile_pool(name="ot_p", bufs=mT + 2) as ot_p,
        ):
            wh2e_t = []
            for i in range(kH):
                t = wh2e_p.tile([P, E], F32R, tag=f"wh2e{i}", name=f"wh2et{i}")
                nc.sync.dma_start(t, d_wh2e[P * i : P * (i + 1), :])
                wh2e_t.append(t)
            co = []
            for m in range(kE):
                ps = pp.tile([P, T], F32, tag="ps", name=f"cops{m}")
                for k in range(kH):
                    nc.tensor.matmul(
                        ps,
                        wh2e_t[k][:, P * m : P * (m + 1)],
                        u[k][:, KW - 1 :],
                        start=(k == 0),
                        stop=(k == kH - 1 and not with_bias),
                    )
                if with_bias:
                    nc.tensor.matmul(
                        ps,
                        bh2e_t[:, P * m : P * (m + 1)],
                        ones_row,
                        start=False,
                        stop=True,
                    )
                t = co_p.tile([P, T], BF16, tag=f"co{m}", name=f"co{m}")
                nc.scalar.copy(t, ps)
                co.append(t)

            # chunk groups of GS: bigger DMA transfers for fcw reads and
            # output writes (4x inner-contig), psum stays one CH-chunk
            GS = 4 if NCH % 4 == 0 else (2 if NCH % 2 == 0 else 1)
            GW = GS * CH
            for cg in range(NCH // GS):
                fts = []
                for k in range(kE):
                    ft = fcw_p.tile([P, GW], BF16, tag="fcw", name=f"fcw{cg}_{k}")
                    nc.sync.dma_start(
                        ft, d_fcw[P * k : P * (k + 1), GW * cg : GW * (cg + 1)]
                    )
                    fts.append(ft)
                for m in range(mT):
                    ot = ot_p.tile([P, GW], F32, tag="ot", name=f"ot{cg}_{m}")
                    for sub in range(GS):
                        ps = pp.tile([P, CH], F32, tag="ps", name=f"fcps{cg}_{m}_{sub}")
                        for k in range(kE):
                            nc.tensor.matmul(
                                ps,
                                co[k][:, P * m : P * (m + 1)],
                                fts[k][:, CH * sub : CH * (sub + 1)],
                                start=(k == 0),
                                stop=(k == kE - 1),
                            )
                        nc.vector.tensor_copy(ot[:, CH * sub : CH * (sub + 1)], ps)
                    nc.sync.dma_start(
                        d_out[P * m : P * (m + 1), GW * cg : GW * (cg + 1)], ot
                    )

    if legalize:
        _legalize_pe_waits(nc)
    return nc


def _host_prep(inp, T, L, KW):
    """Host-side input prep shared by kernel() and tests: embedding gather,
    transposes, conv-weight relayout."""
    f32 = np.float32
    trg = np.asarray(inp["trg"]).astype(np.int64)
    tok = np.asarray(inp["tok_emb"], dtype=f32)
    pos = np.asarray(inp["pos_emb"], dtype=f32)
    embedded = tok[trg] + pos[:T][None]  # (B,T,E)
    sq = f32(np.sqrt(np.float32(0.5)))
    embT = np.ascontiguousarray(embedded.transpose(0, 2, 1))
    embsT = np.ascontiguousarray((embedded * sq).transpose(0, 2, 1))
    encT = np.ascontiguousarray(
        np.asarray(inp["encoder_conved"], dtype=f32).transpose(0, 2, 1)
    )
    encC = np.ascontiguousarray(np.asarray(inp["encoder_combined"], dtype=f32))
    import ml_dtypes

    cw = np.ascontiguousarray(
        np.asarray(inp["conv_w"], dtype=f32).transpose(0, 3, 2, 1)
    ).astype(ml_dtypes.bfloat16)  # (L, KW, H, 2H) bf16
    return embT, embsT, encT, encC, cw


def kernel(**inputs):
    B, T, S = 8, 512, 512
    E, H, V = 512, 1024, 32000
    KW, L = 3, 6
    CH = 500

    import ml_dtypes

    f32 = np.float32
    inp = {k: np.asarray(v) for k, v in inputs.items()}
    embT, embsT, encT, encC, cw = _host_prep(inp, T, L, KW)

    dev_biases = ["emb2hid_b", "conv_b", "attn_hid2emb_b", "attn_emb2hid_b", "hid2emb_b"]
    with_bias = any(np.any(np.asarray(inp[k])) for k in dev_biases)

    nc = build_decoder_nc(
        T=T, S=S, E=E, H=H, V=V, L=L, KW=KW, CH=CH, with_bias=with_bias
    )

    base = {
        "c_pad": np.full((128, KW - 1), f32(1.0)),
        "c_ones_col": np.ones((128, 1), f32),
        "c_halfones": np.full((1, 128), f32(0.5)),
        "we2h": np.ascontiguousarray(np.asarray(inp["emb2hid_w"], dtype=f32)),
        "w1": np.ascontiguousarray(np.asarray(inp["attn_hid2emb_w"], dtype=f32)),
        "w2": np.ascontiguousarray(np.asarray(inp["attn_emb2hid_w"], dtype=f32)),
        "wh2e": np.ascontiguousarray(np.asarray(inp["hid2emb_w"], dtype=f32)),
        "fcw": np.ascontiguousarray(np.asarray(inp["fc_out_w"], dtype=f32)).astype(
            ml_dtypes.bfloat16
        ),
        "cw": cw,
    }
    if with_bias:
        base |= {
            "c_ones_row": np.ones((1, T), f32),
            "b_e2h": np.asarray(inp["emb2hid_b"], dtype=f32).reshape(1, H),
            "b1": np.asarray(inp["attn_hid2emb_b"], dtype=f32).reshape(1, E),
            "b2s2": (np.asarray(inp["attn_emb2hid_b"], dtype=f32) * f32(0.5)).reshape(H, 1),
            "bh2e": np.asarray(inp["hid2emb_b"], dtype=f32).reshape(1, E),
            "cb_bf": np.ascontiguousarray(np.asarray(inp["conv_b"], dtype=f32)).astype(
                ml_dtypes.bfloat16
            ),
        }
    in_maps = [
        dict(base, embT=embT[c], embsT=embsT[c], encT=encT[c], encC=encC[c])
        for c in range(B)
    ]

    from concourse.bass_utils import run_bass_kernel_spmd

    import os

    trace = bool(os.environ.get("DECODER_TRACE"))
    res = run_bass_kernel_spmd(nc, in_maps, core_ids=list(range(B)), trace=trace)
    global _last_results
    _last_results = res
    out = np.stack([res.results[c]["out"] for c in range(B)]).astype(f32)

    fcb = np.asarray(inp["fc_out_b"], dtype=f32)
    if np.any(fcb):
        out = out + fcb[None, None, :]
    return out


# revision 54
# speedup vs baseline: 1.4794x; 1.0065x over previous
"""Trainium2 Bass kernel for the ConvS2S-style decoder (nn_Decoder).

Strategy: pure data-parallel over batch — B=8 batch elements mapped 1:1 onto
8 NeuronCores, zero cross-core communication.  Host does the (tiny) embedding
gather + a few weight transposes; each core runs the full per-batch pipeline:

    u = emb @ W_e2h                                (H,T layout, left-pad cols = 1.0)
    6 x [ conv(K=3, via 3 shifted matmuls) -> GLU
          -> attention (energy in (S,T) layout, softmax over partition dim via
             ones-matmul column sums, normalization folded in as a rank-1
             broadcast matmul) -> residual updates ]
    convout = u.T @ W_h2e ; out = convout @ W_fc   (T,V layout, 64 V-chunks)

All matmuls run as float32r (fp32 storage, fp22 multiply) which is full PE
rate for moving-dim >= 256; every tensor feeding a matmul is typed float32r
end-to-end (walrus requires matmul inputs be produced pre-rounded).
Activations stay in SBUF for the whole layer stack; only weights stream from
HBM.
"""

import numpy as np
from contextlib import ExitStack

import bass_rust
import concourse.bass as bass
import concourse.mybir as mybir
import concourse.tile as tile
from concourse.alu_op_type import AluOpType

F32 = mybir.dt.float32
F32R = mybir.dt.float32r
BF16 = mybir.dt.bfloat16
AF = mybir.ActivationFunctionType
P = 128

_last_results = None


def _legalize_pe_waits(nc):
    """Walrus packs a self-loading (fp32/fp32r) Matmult's sync waits into the
    LDWEIGHTS hw descriptor, which has a single wait slot.  Move the waits of
    any multi-wait PE compute instruction onto EventSemaphore instructions
    (one wait each) inserted just before it on the PE queue — semantically
    identical wait point, but each carrier is within the hw limit."""
    n = 0
    absorb_types = (
        "InstMatmult",
        "InstLdweights",
        "InstDMACopy",
        "InstActivation",
        "InstTensorTensor",
        "InstTensorScalarPtr",
        "InstTensorCopy",
        "InstReciprocal",
        "InstMemset",
        "InstTensorReduce",
        "InstDrain",
    )
    for fn in nc.m.functions:
        for blk in fn.blocks:
            out = []
            changed = False
            for inst in blk.instructions:
                si = inst.sync_info
                if (
                    si is not None
                    and type(inst).__name__ in absorb_types
                ):
                    waits = list(si.on_wait)
                    if len(waits) > 1:
                        for w in waits:
                            out.append(
                                mybir.InstEventSemaphore(
                                    name=f"I-pewait{n}",
                                    engine=inst.engine,
                                    sync_info=bass_rust.SyncInfo(
                                        on_wait=[w], on_update=[]
                                    ),
                                    ins=[],
                                    outs=[],
                                )
                            )
                            n += 1
                        inst.sync_info = bass_rust.SyncInfo(
                            on_wait=[], on_update=list(si.on_update)
                        )
                        changed = True
                out.append(inst)
            if changed:
                blk.instructions = out
    return n


def build_decoder_nc(T, S, E, H, V, L, KW, CH, with_bias, pad_val=1.0, legalize=True):
    """Build the per-core Bass program.  All dims must be multiples of 128
    (except V which must be a multiple of CH, CH <= 512)."""
    kE, kH, kS, mT = E // P, H // P, S // P, T // P
    NCH = V // CH
    SQ = float(np.sqrt(np.float32(0.5)))
    S2 = 0.5  # SQ**2 exactly

    nc = bass.Bass()

    d_embT = nc.declare_dram_parameter("embT", [E, T], F32R, isOutput=False)
    d_embsT = nc.declare_dram_parameter("embsT", [E, T], F32, isOutput=False)
    d_encT = nc.declare_dram_parameter("encT", [E, S], F32R, isOutput=False)
    d_encC = nc.declare_dram_parameter("encC", [S, E], F32R, isOutput=False)
    d_we2h = nc.declare_dram_parameter("we2h", [E, H], F32R, isOutput=False)
    d_w1 = nc.declare_dram_parameter("w1", [H, E], F32R, isOutput=False)
    d_w2 = nc.declare_dram_parameter("w2", [E, H], F32R, isOutput=False)
    d_wh2e = nc.declare_dram_parameter("wh2e", [H, E], F32R, isOutput=False)
    d_fcw = nc.declare_dram_parameter("fcw", [E, V], BF16, isOutput=False)
    d_cw = nc.declare_dram_parameter("cw", [L, KW, H, 2 * H], BF16, isOutput=False)
    d_cpad = nc.declare_dram_parameter("c_pad", [P, KW - 1], F32R, isOutput=False)
    d_cones = nc.declare_dram_parameter("c_ones_col", [P, 1], F32R, isOutput=False)
    d_chalf = nc.declare_dram_parameter("c_halfones", [1, P], F32R, isOutput=False)
    if with_bias:
        d_be2h = nc.declare_dram_parameter("b_e2h", [1, H], F32R, isOutput=False)
        d_b1 = nc.declare_dram_parameter("b1", [1, E], F32R, isOutput=False)
        d_b2s2 = nc.declare_dram_parameter("b2s2", [H, 1], F32, isOutput=False)
        d_bh2e = nc.declare_dram_parameter("bh2e", [1, E], F32R, isOutput=False)
    d_out = nc.declare_dram_parameter("out", [T, V], F32, isOutput=True)

    with tile.TileContext(nc) as tc, ExitStack() as ctx:
        pers = ctx.enter_context(tc.tile_pool(name="pers", bufs=1))
        pp = ctx.enter_context(tc.tile_pool(name="pp", bufs=8, space="PSUM"))

        # ---- persistent SBUF tensors -------------------------------------
        u = []
        for i in range(kH):
            t = pers.tile([P, T + KW - 1], F32R, tag=f"u{i}", name=f"u{i}")
            u.append(t)
            nc.sync.dma_start(t[:, 0 : KW - 1], d_cpad[:, :])
        # DMA emission order = rough hardware arrival order; the init-phase
        # inputs (embT/we2h, emitted in the init block below) must land first
        # so PE starts ~10us in — so the persistent tensors are DMA'd from
        # this deferred hook, called after the init DMAs are emitted.
        embs_t = [
            pers.tile([P, T], F32, tag=f"embs{i}", name=f"embs{i}") for i in range(kE)
        ]
        encT_t = [
            pers.tile([P, S], F32R, tag=f"encT{i}", name=f"encTt{i}")
            for i in range(kE)
        ]
        encC_t = [
            pers.tile([P, E], F32R, tag=f"encC{i}", name=f"encCt{i}")
            for i in range(kS)
        ]
        w1_t = [
            pers.tile([P, E], F32R, tag=f"w1_{i}", name=f"w1t{i}") for i in range(kH)
        ]
        w2_t = [
            pers.tile([P, H], F32R, tag=f"w2_{i}", name=f"w2t{i}") for i in range(kE)
        ]
        ones_col = pers.tile([P, 1], F32R, tag="ones_col", name="ones_col")
        halfones = pers.tile([1, P], F32R, tag="halfones", name="halfones")

        def _dma_persistent():
            for i in range(kH):
                nc.sync.dma_start(w1_t[i], d_w1[P * i : P * (i + 1), :])
            for i in range(kE):
                nc.sync.dma_start(embs_t[i], d_embsT[P * i : P * (i + 1), :])
            for i in range(kE):
                nc.sync.dma_start(encT_t[i], d_encT[P * i : P * (i + 1), :])
            for i in range(kS):
                nc.sync.dma_start(encC_t[i], d_encC[P * i : P * (i + 1), :])
            for i in range(kE):
                nc.sync.dma_start(w2_t[i], d_w2[P * i : P * (i + 1), :])
            nc.sync.dma_start(ones_col, d_cones[:, :])
            nc.sync.dma_start(halfones, d_chalf[:, :])

        if with_bias:
            d_crow = nc.declare_dram_parameter("c_ones_row", [1, T], F32R, isOutput=False)
            ones_row = pers.tile([1, T], F32R, tag="ones_row", name="ones_row")
            nc.sync.dma_start(ones_row, d_crow[:, :])
            ones_row_bf = pers.tile([1, T], BF16, tag="ones_row_bf", name="ones_row_bf")
            nc.vector.tensor_copy(ones_row_bf, ones_row)
            be2h_t = pers.tile([1, H], F32R, tag="be2h", name="be2h_t")
            nc.sync.dma_start(be2h_t, d_be2h[:, :])
            b1_t = pers.tile([1, E], F32R, tag="b1", name="b1_t")
            nc.sync.dma_start(b1_t, d_b1[:, :])
            b2s2_sb = []
            for m in range(kH):
                t = pers.tile([P, 1], F32, tag=f"b2s2_{m}", name=f"b2s2_{m}")
                nc.sync.dma_start(t, d_b2s2[P * m : P * (m + 1), :])
                b2s2_sb.append(t)
            bh2e_t = pers.tile([1, E], F32R, tag="bh2e", name="bh2e_t")
            nc.sync.dma_start(bh2e_t, d_bh2e[:, :])
            d_cbf = nc.declare_dram_parameter("cb_bf", [L, 2 * H], BF16, isOutput=False)
            cb_t = []
            for l in range(L):
                t = pers.tile([1, 2 * H], BF16, tag=f"cb{l}", name=f"cb_t{l}")
                nc.sync.dma_start(t, d_cbf[l : l + 1, :])
                cb_t.append(t)

        # ---- init: u[:, KW-1:] = W_e2h.T @ embT (+ b_e2h) ----------------
        ubf_pers = ctx.enter_context(tc.tile_pool(name="ubf_p", bufs=kH))
        ubf = []
        with tc.tile_pool(name="initp", bufs=1) as initp:
            embT_t = []
            for i in range(kE):
                t = initp.tile([P, T], F32R, tag=f"embT{i}", name=f"embTt{i}")
                nc.sync.dma_start(t, d_embT[P * i : P * (i + 1), :])
                embT_t.append(t)
            we2h_t = []
            for i in range(kE):
                t = initp.tile([P, H], F32R, tag=f"we2h{i}", name=f"we2ht{i}")
                nc.sync.dma_start(t, d_we2h[P * i : P * (i + 1), :])
                we2h_t.append(t)
            _dma_persistent()
            for m in range(kH):
                ps = pp.tile([P, T], F32, tag="ps", name=f"initps{m}")
                for k in range(kE):
                    nc.tensor.matmul(
                        ps,
                        we2h_t[k][:, P * m : P * (m + 1)],
                        embT_t[k],
                        start=(k == 0),
                        stop=(k == kE - 1 and not with_bias),
                    )
                if with_bias:
                    nc.tensor.matmul(
                        ps,
                        be2h_t[:, P * m : P * (m + 1)],
                        ones_row,
                        start=False,
                        stop=True,
                    )
                nc.scalar.copy(u[m][:, KW - 1 :], ps)
                t = ubf_pers.tile([P, T + KW - 1], BF16, tag="ubf", name=f"ubf0_{m}")
                nc.scalar.copy(t, u[m])
                ubf.append(t)

        # ---- layer stack -------------------------------------------------
        n_stripes = KW * kH
        with (
            tc.tile_pool(name="wconv_p", bufs=n_stripes + 8) as wconv_p,
            tc.tile_pool(name="sig_p", bufs=kH) as sig_p,
            tc.tile_pool(name="glu_p", bufs=kH) as glu_p,
            tc.tile_pool(name="comb_p", bufs=kE) as comb_p,
            tc.tile_pool(name="ex_p", bufs=kS) as ex_p,
            tc.tile_pool(name="att_p", bufs=kE) as att_p,
            tc.tile_pool(name="rec_p", bufs=1) as rec_p,
            tc.tile_pool(name="y_p", bufs=2) as y_p,
        ):
            for l in range(L):
                u_bf = ubf
                # conv + GLU: g-half (gate) first, then a-half; m-outer with
                # the half's full weight set resident so each psum finishes
                # early and GLU/attention overlap the remaining conv matmuls.
                sig = []
                glu = []
                for half in (1, 0):  # 1 = gate channels [H:2H), 0 = a [0:H)
                    wsts = []
                    for kw in range(KW):
                        for k in range(kH):
                            wst = wconv_p.tile(
                                [P, H], BF16, tag="wst", name=f"wst{l}_{half}_{kw}_{k}"
                            )
                            nc.sync.dma_start(
                                wst,
                                d_cw[l, kw, P * k : P * (k + 1), H * half : H * (half + 1)],
                            )
                            wsts.append((kw, k, wst))
                    for m in range(kH):
                        cps = pp.tile([P, T], F32, tag="ps", name=f"cps{l}_{half}_{m}")
                        for i_mm, (kw, k, wst) in enumerate(wsts):
                            nc.tensor.matmul(
                                cps,
                                wst[:, P * m : P * (m + 1)],
                                u_bf[k][:, kw : kw + T],
                                start=(i_mm == 0),
                                stop=(i_mm == n_stripes - 1 and not with_bias),
                            )
                        if with_bias:
                            nc.tensor.matmul(
                                cps,
                                cb_t[l][
                                    :, half * H + P * m : half * H + P * (m + 1)
                                ],
                                ones_row_bf,
                                start=False,
                                stop=True,
                            )
                        if half == 1:
                            sg = sig_p.tile([P, T], BF16, tag="sig", name=f"sig{l}_{m}")
                            nc.scalar.activation(sg, cps, AF.Sigmoid)
                            sig.append(sg)
                        else:
                            g = glu_p.tile([P, T], F32R, tag="glu", name=f"glu{l}_{m}")
                            nc.vector.tensor_mul(g, cps, sig[m])
                            glu.append(g)

                # attention: combined = (glu.T @ w1 (+b1)) * s + emb*s, (E,T)
                comb = []
                for m in range(kE):
                    ps = pp.tile([P, T], F32, tag="ps", name=f"ceps{l}_{m}")
                    for k in range(kH):
                        nc.tensor.matmul(
                            ps,
                            w1_t[k][:, P * m : P * (m + 1)],
                            glu[k],
                            start=(k == 0),
                            stop=(k == kH - 1 and not with_bias),
                        )
                    if with_bias:
                        nc.tensor.matmul(
                            ps,
                            b1_t[:, P * m : P * (m + 1)],
                            ones_row,
                            start=False,
                            stop=True,
                        )
                    c = comb_p.tile([P, T], F32R, tag="comb", name=f"comb{l}_{m}")
                    nc.vector.scalar_tensor_tensor(
                        c, ps, SQ, embs_t[m], AluOpType.mult, AluOpType.add
                    )
                    comb.append(c)

                # energy in (S, T) layout; exp elementwise (energies are
                # bounded ~|22| for this model, fp32-safe without max-sub)
                ex = []
                for m in range(kS):
                    ps = pp.tile([P, T], F32, tag="ps", name=f"enps{l}_{m}")
                    for k in range(kE):
                        nc.tensor.matmul(
                            ps,
                            encT_t[k][:, P * m : P * (m + 1)],
                            comb[k],
                            start=(k == 0),
                            stop=(k == kE - 1),
                        )
                    e = ex_p.tile([P, T], F32R, tag="ex", name=f"ex{l}_{m}")
                    nc.scalar.activation(e, ps, AF.Exp)
                    ex.append(e)

                # column sums over S (partition dim) via ones matmul; then
                # rec_bc[p, t] = 0.5 / sums[t] broadcast to 128 partitions
                sps = pp.tile([1, T], F32, tag="ps", name=f"sums{l}")
                for k in range(kS):
                    nc.tensor.matmul(
                        sps, ones_col, ex[k], start=(k == 0), stop=(k == kS - 1)
                    )
                rec = rec_p.tile([1, T], F32R, tag="rec", name=f"rec{l}")
                with nc.allow_low_precision(reason="softmax recip feeds f32r matmul"):
                    nc.vector.reciprocal(rec, sps)
                bps = pp.tile([P, T], F32, tag="ps", name=f"bps{l}")
                nc.tensor.matmul(bps, halfones, rec, start=True, stop=True)
                rbc = rec_p.tile([P, T], F32, tag="rbc", name=f"rbc{l}")
                nc.scalar.copy(rbc, bps)

                # attended (E,T), unnormalized — normalization (x rbc) is
                # applied after the att2 matmul so the reciprocal chain
                # overlaps PE work instead of stalling it
                att = []
                for m in range(kE):
                    ps = pp.tile([P, T], F32, tag="ps", name=f"atps{l}_{m}")
                    for k in range(kS):
                        nc.tensor.matmul(
                            ps,
                            encC_t[k][:, P * m : P * (m + 1)],
                            ex[k],
                            start=(k == 0),
                            stop=(k == kS - 1),
                        )
                    a = att_p.tile([P, T], F32R, tag="att", name=f"att{l}_{m}")
                    nc.scalar.copy(a, ps)
                    att.append(a)

                # att2 = w2.T @ att (+ b2*s^2/rbc); then per m-tile:
                #   x1 = att2_psum * rbc          (DVE, psum operand)
                #   y  = glu*s^2 + x1             (GPSIMD, sbuf only)
                #   u  = u*s + y                  (DVE)
                #   ubf= bf16(u)                  (ACT) -> next layer's conv
                next_ubf = []
                for m in range(kH):
                    ps = pp.tile([P, T], F32, tag="ps", name=f"a2ps{l}_{m}")
                    for k in range(kE):
                        nc.tensor.matmul(
                            ps,
                            w2_t[k][:, P * m : P * (m + 1)],
                            att[k],
                            start=(k == 0),
                            stop=(k == kE - 1),
                        )
                    x1 = y_p.tile([P, T], F32, tag="x1", name=f"x1_{l}_{m}")
                    nc.vector.tensor_mul(x1, ps, rbc)
                    if with_bias:
                        nc.vector.tensor_scalar_add(
                            x1, x1, b2s2_sb[m]
                        )
                    y = y_p.tile([P, T], F32, tag="y", name=f"y{l}_{m}")
                    nc.vector.scalar_tensor_tensor(
                        y, glu[m], S2, x1, AluOpType.mult, AluOpType.add
                    )
                    nc.vector.scalar_tensor_tensor(
                        u[m][:, KW - 1 :],
                        u[m][:, KW - 1 :],
                        SQ,
                        y,
                        AluOpType.mult,
                        AluOpType.add,
                    )
                    nb = ubf_pers.tile(
                        [P, T + KW - 1], BF16, tag="ubf", name=f"ubf{l + 1}_{m}"
                    )
                    nc.scalar.copy(nb, u[m])
                    next_ubf.append(nb)
                ubf = next_ubf

        # ---- final: convout (E,T) then fc_out (T,V) ----------------------
        with (
            tc.tile_pool(name="wh2e_p", bufs=1) as wh2e_p,
            tc.tile_pool(name="co_p", bufs=1) as co_p,
            tc.tile_pool(name="fcw_p", bufs=3 * kE) as fcw_p,
            tc.tile_pool(name="ot_p", bufs=mT + 2) as ot_p,
        ):
            wh2e_t = []
            for i in range(kH):
                t = wh2e_p.tile([P, E], F32R, tag=f"wh2e{i}", name=f"wh2et{i}")
                nc.sync.dma_start(t, d_wh2e[P * i : P * (i + 1), :])
                wh2e_t.append(t)
            co = []
            for m in range(kE):
                ps = pp.tile([P, T], F32, tag="ps", name=f"cops{m}")
                for k in range(kH):
                    nc.tensor.matmul(
                        ps,
                        wh2e_t[k][:, P * m : P * (m + 1)],
                        u[k][:, KW - 1 :],
                        start=(k == 0),
                        stop=(k == kH - 1 and not with_bias),
                    )
                if with_bias:
                    nc.tensor.matmul(
                        ps,
                        bh2e_t[:, P * m : P * (m + 1)],
                        ones_row,
                        start=False,
                        stop=True,
                    )
                t = co_p.tile([P, T], BF16, tag=f"co{m}", name=f"co{m}")
                nc.scalar.copy(t, ps)
                co.append(t)

            # chunk groups of GS: bigger DMA transfers for fcw reads and
            # output writes (4x inner-contig), psum stays one CH-chunk
            GS = 4 if NCH % 4 == 0 else (2 if NCH % 2 == 0 else 1)
            GW = GS * CH
            for cg in range(NCH // GS):
                fts = []
                for k in range(kE):
                    ft = fcw_p.tile([P, GW], BF16, tag="fcw", name=f"fcw{cg}_{k}")
                    nc.sync.dma_start(
                        ft, d_fcw[P * k : P * (k + 1), GW * cg : GW * (cg + 1)]
                    )
                    fts.append(ft)
                for m in range(mT):
                    ot = ot_p.tile([P, GW], F32, tag="ot", name=f"ot{cg}_{m}")
                    for sub in range(GS):
                        ps = pp.tile([P, CH], F32, tag="ps", name=f"fcps{cg}_{m}_{sub}")
                        for k in range(kE):
                            nc.tensor.matmul(
                                ps,
                                co[k][:, P * m : P * (m + 1)],
                                fts[k][:, CH * sub : CH * (sub + 1)],
                                start=(k == 0),
                                stop=(k == kE - 1),
                            )
                        nc.vector.tensor_copy(ot[:, CH * sub : CH * (sub + 1)], ps)
                    nc.sync.dma_start(
                        d_out[P * m : P * (m + 1), GW * cg : GW * (cg + 1)], ot
                    )

    if legalize:
        _legalize_pe_waits(nc)
    return nc


def _host_prep(inp, T, L, KW):
    """Host-side input prep shared by kernel() and tests: embedding gather,
    transposes, conv-weight relayout."""
    f32 = np.float32
    trg = np.asarray(inp["trg"]).astype(np.int64)
    tok = np.asarray(inp["tok_emb"], dtype=f32)
    pos = np.asarray(inp["pos_emb"], dtype=f32)
    embedded = tok[trg] + pos[:T][None]  # (B,T,E)
    sq = f32(np.sqrt(np.float32(0.5)))
    embT = np.ascontiguousarray(embedded.transpose(0, 2, 1))
    embsT = np.ascontiguousarray((embedded * sq).transpose(0, 2, 1))
    encT = np.ascontiguousarray(
        np.asarray(inp["encoder_conved"], dtype=f32).transpose(0, 2, 1)
    )
    encC = np.ascontiguousarray(np.asarray(inp["encoder_combined"], dtype=f32))
    import ml_dtypes

    cw = np.ascontiguousarray(
        np.asarray(inp["conv_w"], dtype=f32).transpose(0, 3, 2, 1)
    ).astype(ml_dtypes.bfloat16)  # (L, KW, H, 2H) bf16
    return embT, embsT, encT, encC, cw


def kernel(**inputs):
    B, T, S = 8, 512, 512
    E, H, V = 512, 1024, 32000
    KW, L = 3, 6
    CH = 500

    import ml_dtypes

    f32 = np.float32
    inp = {k: np.asarray(v) for k, v in inputs.items()}
    embT, embsT, encT, encC, cw = _host_prep(inp, T, L, KW)

    dev_biases = ["emb2hid_b", "conv_b", "attn_hid2emb_b", "attn_emb2hid_b", "hid2emb_b"]
    with_bias = any(np.any(np.asarray(inp[k])) for k in dev_biases)

    nc = build_decoder_nc(
        T=T, S=S, E=E, H=H, V=V, L=L, KW=KW, CH=CH, with_bias=with_bias
    )

    base = {
        "c_pad": np.full((128, KW - 1), f32(1.0)),
        "c_ones_col": np.ones((128, 1), f32),
        "c_halfones": np.full((1, 128), f32(0.5)),
        "we2h": np.ascontiguousarray(np.asarray(inp["emb2hid_w"], dtype=f32)),
        "w1": np.ascontiguousarray(np.asarray(inp["attn_hid2emb_w"], dtype=f32)),
        "w2": np.ascontiguousarray(np.asarray(inp["attn_emb2hid_w"], dtype=f32)),
        "wh2e": np.ascontiguousarray(np.asarray(inp["hid2emb_w"], dtype=f32)),
        "fcw": np.ascontiguousarray(np.asarray(inp["fc_out_w"], dtype=f32)).astype(
            ml_dtypes.bfloat16
        ),
        "cw": cw,
    }
    if with_bias:
        base |= {
            "c_ones_row": np.ones((1, T), f32),
            "b_e2h": np.asarray(inp["emb2hid_b"], dtype=f32).reshape(1, H),
            "b1": np.asarray(inp["attn_hid2emb_b"], dtype=f32).reshape(1, E),
            "b2s2": (np.asarray(inp["attn_emb2hid_b"], dtype=f32) * f32(0.5)).reshape(H, 1),
            "bh2e": np.asarray(inp["hid2emb_b"], dtype=f32).reshape(1, E),
            "cb_bf": np.ascontiguousarray(np.asarray(inp["conv_b"], dtype=f32)).astype(
                ml_dtypes.bfloat16
            ),
        }
    in_maps = [
        dict(base, embT=embT[c], embsT=embsT[c], encT=encT[c], encC=encC[c])
        for c in range(B)
    ]

    from concourse.bass_utils import run_bass_kernel_spmd

    import os

    trace = bool(os.environ.get("DECODER_TRACE"))
    res = run_bass_kernel_spmd(nc, in_maps, core_ids=list(range(B)), trace=trace)
    global _last_results
    _last_results = res
    out = np.stack([res.results[c]["out"] for c in range(B)]).astype(f32)

    fcb = np.asarray(inp["fc_out_b"], dtype=f32)
    if np.any(fcb):
        out = out + fcb[None, None, :]
    return out
